# revision 20
# baseline (speedup 1.0000x reference)
"""Trainium2 Bass kernel for a dense Mamba (selective-scan) block, SPMD over 8 NeuronCores.

Sharding: tensor-parallel over d_inner (2048 -> 256 channels/core).

v2: fully software-pipelined over 8 rounds of 1024 timesteps each:
  round k: AllReduce(k) | in_proj+conv+x_proj for round k+1 (PE/Scalar) |
           dt_proj+softplus (B) | selective scan (C, DVE-bound) |
           tiny AllToAll(k) | out_proj for t-block k (E, PE).
Output t-rows are interleaved in 128-blocks across cores so each round's
gated activations can be exchanged and out_proj'd immediately, hiding the
collective+out_proj tail under the scan. Activations (xs, silu(z), dts,
dtx) stay in SBUF rings; out_proj weights stay resident in SBUF.
DVE runs only the scan chain (exp on Scalar via act tables incl. native
Silu; PSUM->SBUF copies and casts on Scalar; gate epilogue fused via
scalar_tensor_tensor reading PSUM directly).

Shapes hardcoded for: B=2, L=4096, d_model=1024, d_inner=2048, d_state=16,
d_conv=4, dt_rank=64, f32 I/O.
"""
import numpy as np
import ml_dtypes
from contextlib import ExitStack

import concourse.bass as bass
import concourse.bacc as bacc
import concourse.tile as tile
from concourse import mybir
from concourse import bass_utils

BF = ml_dtypes.bfloat16
F32 = mybir.dt.float32
BF16 = mybir.dt.bfloat16

NCORES = 8
B, L, DM = 2, 4096, 1024
DI, DS, DC, DTR = 2048, 16, 4, 64
DL = DI // NCORES          # 256 local channels
NDH = DL // 128            # 2 d-half tiles
T = B * L                  # 8192 flattened (b, l)
TCB = 1024                 # round granularity (t per round)
NR = T // TCB              # 8 rounds
TCA = 512                  # in_proj chunk
ACT = mybir.ActivationFunctionType
ALU = mybir.AluOpType

_cached = {}


def _build():
    nc = bacc.Bacc("TRN2", target_bir_lowering=False, num_devices=NCORES)

    # ---- I/O -------------------------------------------------------------
    d_hT = nc.dram_tensor("hT", (DM, T), BF16, kind="ExternalInput")
    d_wxzT = nc.dram_tensor("wxzT", (DM, 2 * DL), BF16, kind="ExternalInput")
    d_cdiag = nc.dram_tensor("cdiag", (DC, NDH, 128, 128), BF16, kind="ExternalInput")
    d_convb = nc.dram_tensor("convb", (NDH, 128, 1), F32, kind="ExternalInput")
    d_xprojT = nc.dram_tensor("xprojT", (NDH, 128, DTR + 2 * DS), BF16, kind="ExternalInput")
    d_dtwT = nc.dram_tensor("dtwT", (DTR, DL), BF16, kind="ExternalInput")
    d_dtb = nc.dram_tensor("dtb", (NDH, 128, 1), F32, kind="ExternalInput")
    d_aneg = nc.dram_tensor("aneg", (NDH, 128, DS), F32, kind="ExternalInput")
    d_dvec = nc.dram_tensor("dvec", (NDH, 128, 1), F32, kind="ExternalInput")
    d_woutT = nc.dram_tensor("woutT", (2 * NCORES, 128, DM), BF16, kind="ExternalInput")
    d_ident = nc.dram_tensor("ident", (128, 128), BF16, kind="ExternalInput")
    d_out = nc.dram_tensor("out_slice", (NR * 128, DM), F32, kind="ExternalOutput")

    # ---- internal DRAM ---------------------------------------------------
    d_xdp = nc.dram_tensor("xdp", (2 * NR, DTR + 2 * DS, 512), F32, kind="Internal")
    d_xd = nc.dram_tensor("xd", (2 * NR, DTR + 2 * DS, 512), F32, kind="Internal",
                          addr_space="Shared")
    d_bc = nc.dram_tensor("bcrows", (2 * DS, T), BF16, kind="Internal")
    # tiny warm-up collective to absorb initial inter-core skew during A(0)
    d_ri = nc.dram_tensor("warm_i", (4, 64), F32, kind="Internal")
    d_ro = nc.dram_tensor("warm_o", (4, 64), F32, kind="Internal",
                          addr_space="Shared")
    # per-round A2A pieces: [round][target core][local ch][128 t]
    d_a2ai = nc.dram_tensor("a2ai", (NR, NCORES, DL, 128), BF16, kind="Internal")
    d_a2ao = nc.dram_tensor("a2ao", (NR, NCORES, DL, 128), BF16, kind="Internal")

    groups = [list(range(NCORES))]

    with tile.TileContext(nc) as tc, ExitStack() as ctx:
        consts = ctx.enter_context(tc.tile_pool(name="consts", bufs=1))
        arena = ctx.enter_context(tc.tile_pool(name="arena", bufs=1))
        work = ctx.enter_context(tc.tile_pool(name="work", bufs=2))
        psA = ctx.enter_context(tc.tile_pool(name="psA", bufs=2, space="PSUM"))
        psY = ctx.enter_context(tc.tile_pool(name="psY", bufs=4, space="PSUM"))
        psE = ctx.enter_context(tc.tile_pool(name="psE", bufs=2, space="PSUM"))

        # ---- constants ---------------------------------------------------
        wxz = consts.tile([128, 8, 2 * DL], BF16, tag="wxz")
        nc.sync.dma_start(out=wxz, in_=d_wxzT[:, :].rearrange("(k p) m -> p k m", p=128))
        cdg = consts.tile([128, DC, NDH, 128], BF16, tag="cdg")
        nc.sync.dma_start(
            out=cdg, in_=bass.AP(tensor=d_cdiag[:, :, :, :].tensor, offset=0,
                                 ap=[[128, 128], [NDH * 128 * 128, DC], [128 * 128, NDH], [1, 128]]))
        convb = consts.tile([128, NDH, 1], F32, tag="convb")
        nc.sync.dma_start(out=convb, in_=d_convb[:, :, :].rearrange("h p one -> p h one"))
        xprj = consts.tile([128, NDH, DTR + 2 * DS], BF16, tag="xprj")
        nc.sync.dma_start(out=xprj, in_=d_xprojT[:, :, :].rearrange("h p m -> p h m"))
        dtw = consts.tile([DTR, DL], BF16, tag="dtw")
        nc.sync.dma_start(out=dtw, in_=d_dtwT[:, :])
        dtb = consts.tile([128, NDH, 1], F32, tag="dtb")
        nc.sync.dma_start(out=dtb, in_=d_dtb[:, :, :].rearrange("h p one -> p h one"))
        aneg = consts.tile([128, NDH, DS], F32, tag="aneg")
        nc.sync.dma_start(out=aneg, in_=d_aneg[:, :, :].rearrange("h p n -> p h n"))
        dvec = consts.tile([128, NDH, 1], F32, tag="dvec")
        nc.sync.dma_start(out=dvec, in_=d_dvec[:, :, :].rearrange("h p one -> p h one"))
        ident = consts.tile([128, 128], BF16, tag="ident")
        nc.sync.dma_start(out=ident, in_=d_ident[:, :])
        wout = consts.tile([128, 2 * NCORES, DM], BF16, tag="wout")
        nc.sync.dma_start(out=wout, in_=d_woutT[:, :, :].rearrange("k p m -> p k m"))
        carry = consts.tile([128, NDH, DS], F32, tag="carry")

        # ---- SBUF activation rings --------------------------------------
        xpad = arena.tile([128, NDH, B, 3 + L], BF16, tag="xpad")
        xs_r = arena.tile([128, NDH, 3, TCB], BF16, tag="xs_r")     # ring-3 by round
        zs_r = arena.tile([128, NDH, 3, TCB], BF16, tag="zs_r")     # ring-3 (silu(z))
        dts_r = arena.tile([128, NDH, 2, TCB], BF16, tag="dts_r")   # ring-2
        dtx_r = arena.tile([128, NDH, 2, TCB], BF16, tag="dtx_r")   # ring-2

        for h in range(NDH):
            for b in range(B):
                nc.vector.memset(xpad[:, h, b, 0:3], 0.0)

        # ---- phase bodies ------------------------------------------------
        def phase_A(k):
            """in_proj + z-silu + conv-silu + x_proj partial for round k."""
            b, l0 = (k * TCB) // L, (k * TCB) % L
            r3, r2 = k % 3, k % 2
            for ci in range(TCB // TCA):  # 2 chunks of 512
                t0 = k * TCB + ci * TCA
                lc = l0 + ci * TCA
                ht = work.tile([128, 8, TCA], BF16, tag="ht")
                nc.sync.dma_start(
                    out=ht,
                    in_=bass.AP(tensor=d_hT[:, :].tensor, offset=t0,
                                ap=[[T, 128], [128 * T, 8], [1, TCA]]))
                for m in range(4):  # 0,1: x halves; 2,3: z halves
                    pxz = psA.tile([128, TCA], F32, tag="ps", name=f"pxz_{k}_{ci}_{m}")
                    for kk in range(8):
                        nc.tensor.matmul(pxz, lhsT=wxz[:, kk, m * 128:(m + 1) * 128],
                                         rhs=ht[:, kk, :], start=(kk == 0), stop=(kk == 7))
                    if m < 2:
                        nc.scalar.copy(xpad[:, m, b, 3 + lc: 3 + lc + TCA], pxz)
                    else:
                        nc.scalar.activation(
                            zs_r[:, m - 2, r3, ci * TCA:(ci + 1) * TCA], pxz, ACT.Silu)
                # conv for this chunk (xpad for it was just written)
                for h in range(NDH):
                    pc = psA.tile([128, TCA], F32, tag="ps", name=f"pc_{k}_{ci}_{h}")
                    for j in range(DC):
                        nc.tensor.matmul(pc, lhsT=cdg[:, j, h, :],
                                         rhs=xpad[:, h, b, lc + j: lc + j + TCA],
                                         start=(j == 0), stop=(j == DC - 1))
                    nc.scalar.activation(
                        xs_r[:, h, r3, ci * TCA:(ci + 1) * TCA], pc, ACT.Silu,
                        bias=convb[:, h, 0:1], scale=1.0)
                # x_proj partial for this chunk
                pxp = psA.tile([96, TCA], F32, tag="ps", name=f"pxp_{k}_{ci}")
                for h in range(NDH):
                    nc.tensor.matmul(pxp, lhsT=xprj[:, h, :],
                                     rhs=xs_r[:, h, r3, ci * TCA:(ci + 1) * TCA],
                                     start=(h == 0), stop=(h == NDH - 1))
                xpt = work.tile([96, TCA], F32, tag="xpt")
                nc.scalar.copy(xpt, pxp)
                nc.sync.dma_start(out=d_xdp[2 * k + ci, :, :], in_=xpt)

        def phase_B(k, q0, q1):
            """dt_proj + softplus + dtx for column-halves [q0, q1) of round k."""
            t0 = k * TCB
            r3, r2 = k % 3, k % 2
            spes = []
            for qq in range(q0, q1):
                xdt = work.tile([96, 512], F32, tag="xdt", name=f"xdt_{k}_{qq}")
                nc.sync.dma_start(out=xdt, in_=d_xd[2 * k + qq, :, :])
                xdb = work.tile([96, 512], BF16, tag="xdb", name=f"xdb_{k}_{qq}")
                nc.scalar.copy(xdb, xdt)
                nc.sync.dma_start(out=d_bc[:, t0 + qq * 512:t0 + (qq + 1) * 512],
                                  in_=xdb[DTR:DTR + 2 * DS, :])
                for h in range(NDH):
                    pdt = psA.tile([128, 512], F32, tag="ps", name=f"pdt_{k}_{h}_{qq}")
                    nc.tensor.matmul(pdt, lhsT=dtw[:, h * 128:(h + 1) * 128],
                                     rhs=xdb[0:DTR, :], start=True, stop=True)
                    spe = work.tile([128, 512], F32, tag="spe", bufs=4,
                                    name=f"spe_{k}_{h}_{qq}")
                    nc.scalar.activation(spe, pdt, ACT.Exp,
                                         bias=dtb[:, h, 0:1], scale=1.0)
                    spes.append((h, qq, spe))
            for h, qq, spe in spes:
                nc.scalar.activation(dts_r[:, h, r2, qq * 512:(qq + 1) * 512],
                                     spe, ACT.Ln, bias=1.0, scale=1.0)
            for h in range(NDH):
                nc.vector.tensor_mul(dtx_r[:, h, r2, q0 * 512:q1 * 512],
                                     dts_r[:, h, r2, q0 * 512:q1 * 512],
                                     xs_r[:, h, r3, q0 * 512:q1 * 512])

        def phase_C(k, q0, q1, pys):
            """selective scan for columns [q0*512, q1*512) of round k."""
            c0, W = q0 * 512, (q1 - q0) * 512
            t0c = k * TCB + c0
            r3, r2 = k % 3, k % 2
            first_in_seq = (t0c % L == 0)
            copy_carry = ((t0c + W) % L != 0)
            for j in range(DS // 2):  # n-pairs
                bbc = work.tile([128, 2, W], BF16, tag="bbc",
                                name=f"bbc_{k}_{q0}_{j}")
                cbc = work.tile([128, 2, W], BF16, tag="cbc",
                                name=f"cbc_{k}_{q0}_{j}")
                for nn in range(2):
                    n = 2 * j + nn
                    nc.sync.dma_start(
                        out=bbc[:, nn, :],
                        in_=bass.AP(tensor=d_bc[:, :].tensor, offset=n * T + t0c,
                                    ap=[[0, 128], [1, W]]))
                    nc.sync.dma_start(
                        out=cbc[:, nn, :],
                        in_=bass.AP(tensor=d_bc[:, :].tensor, offset=(DS + n) * T + t0c,
                                    ap=[[0, 128], [1, W]]))
                for h in range(NDH):
                    dA = [None, None]
                    for nn in range(2):
                        n = 2 * j + nn
                        dA[nn] = work.tile([128, W], BF16, tag="dA", bufs=3,
                                           name=f"dA_{k}_{q0}_{j}_{h}_{nn}")
                        nc.scalar.activation(dA[nn], dts_r[:, h, r2, c0:c0 + W],
                                             ACT.Exp, bias=0.0,
                                             scale=aneg[:, h, n:n + 1])
                    dtxs = dtx_r[:, h, r2, c0:c0 + W]
                    dBx = work.tile([128, 2, W], BF16, tag="dBx", bufs=1,
                                    name=f"dBx_{k}_{q0}_{j}_{h}")
                    nc.vector.tensor_mul(
                        dBx,
                        bass.AP(tensor=dtxs.tensor, offset=dtxs.offset,
                                ap=[dtxs.ap[0], [0, 2], dtxs.ap[1]]),
                        bbc)
                    hts = work.tile([128, 2, W], BF16, tag="hts", bufs=1,
                                    name=f"hts_{k}_{q0}_{j}_{h}")
                    for nn in range(2):
                        n = 2 * j + nn
                        init = 0.0 if first_in_seq else carry[:, h, n:n + 1]
                        nc.vector.tensor_tensor_scan(
                            out=hts[:, nn, :], data0=dA[nn], data1=dBx[:, nn, :],
                            initial=init, op0=ALU.mult, op1=ALU.add)
                        if copy_carry:
                            nc.vector.tensor_copy(carry[:, h, n:n + 1],
                                                  hts[:, nn, W - 1:W])
                    yp = work.tile([128, 2, W], BF16, tag="yp", bufs=1,
                                   name=f"yp_{k}_{q0}_{j}_{h}")
                    nc.vector.tensor_mul(yp, hts, cbc)
                    for nn in range(2):
                        for qq in range(q0, q1):
                            nc.tensor.matmul(
                                pys[h][qq], lhsT=ident,
                                rhs=yp[:, nn, (qq - q0) * 512:(qq - q0 + 1) * 512],
                                start=(j == 0 and nn == 0),
                                stop=(j == DS // 2 - 1 and nn == 1))
            if q1 != 2:
                return
            # epilogue: ys = (xs*D + psY) * silu(z); scatter to A2A input
            for h in range(NDH):
                ys = work.tile([128, TCB], BF16, tag="ys", name=f"ys_{k}_{h}")
                for qq in range(2):
                    nc.scalar.copy(ys[:, qq * 512:(qq + 1) * 512], pys[h][qq])
                xsd = work.tile([128, TCB], BF16, tag="xsd", name=f"xsd_{k}_{h}")
                nc.vector.tensor_scalar_mul(xsd, xs_r[:, h, r3, :], dvec[:, h, 0:1])
                nc.vector.tensor_add(ys, ys, xsd)
                nc.vector.tensor_mul(ys, ys, zs_r[:, h, r3, :])
                for jsh in range(NCORES):
                    nc.sync.dma_start(
                        out=d_a2ai[k, jsh, h * 128:(h + 1) * 128, :],
                        in_=ys[:, jsh * 128:(jsh + 1) * 128])

        def phase_E(k):
            """out_proj for my interleaved t-block of round k."""
            yblk = work.tile([128, NCORES, NDH, 128], BF16, tag="yblk")
            nc.sync.dma_start(
                out=yblk,
                in_=bass.AP(tensor=d_a2ao[:, :, :, :].tensor,
                            offset=k * NCORES * DL * 128,
                            ap=[[128, 128], [DL * 128, NCORES], [128 * 128, NDH], [1, 128]]))
            for fh in range(2):
                pe = psE.tile([128, 512], F32, tag="pe", name=f"pe_{k}_{fh}")
                for kt in range(2 * NCORES):
                    i, h = kt // 2, kt % 2
                    nc.tensor.matmul(pe, lhsT=yblk[:, i, h, :],
                                     rhs=wout[:, kt, fh * 512:(fh + 1) * 512],
                                     start=(kt == 0), stop=(kt == 2 * NCORES - 1))
                ot = work.tile([128, 512], F32, tag="ot", bufs=1)
                nc.scalar.copy(ot, pe)
                nc.sync.dma_start(
                    out=d_out[k * 128:(k + 1) * 128, fh * 512:(fh + 1) * 512],
                    in_=ot)

        def all_reduce(i):
            nc.gpsimd.collective_compute(
                kind="AllReduce", op=ALU.add, replica_groups=groups,
                ins=[d_xdp[i, :, :]], outs=[d_xd[i, :, :]])

        def all_to_all(k):
            nc.gpsimd.collective_compute(
                kind="AllToAll", op=ALU.bypass, replica_groups=groups,
                ins=[d_a2ai[k, :, :, :]], outs=[d_a2ao[k, :, :, :]])

        # ---- pipelined schedule -----------------------------------------
        nc.gpsimd.collective_compute(
            kind="AllReduce", op=ALU.add, replica_groups=groups,
            ins=[d_ri[:, :]], outs=[d_ro[:, :]])
        phase_A(0)
        all_reduce(0)
        all_reduce(1)
        for k in range(NR):
            pys = [[psY.tile([128, 512], F32, tag="py", name=f"pys_{k}_{h}_{qq}")
                    for qq in range(2)] for h in range(NDH)]
            if k == 0:
                # round 0: run B/C per half-round to start the scan ASAP,
                # before enqueueing the next round's A work
                phase_B(0, 0, 1)
                phase_C(0, 0, 1, pys)
                phase_A(1)
                all_reduce(2)
                all_reduce(3)
                phase_B(0, 1, 2)
                phase_C(0, 1, 2, pys)
            else:
                if k + 1 < NR:
                    phase_A(k + 1)
                    all_reduce(2 * k + 2)
                    all_reduce(2 * k + 3)
                phase_B(k, 0, 2)
                phase_C(k, 0, 2, pys)
            all_to_all(k)
            phase_E(k)

    nc.compile()
    return nc


def _host_prep(inputs):
    """Per-core input maps from full inputs (layout prep + bf16 casts only)."""
    hs = np.asarray(inputs["hidden_states"], np.float32)
    wxz = np.asarray(inputs["in_proj_w"], np.float32)
    cw = np.asarray(inputs["conv_w"], np.float32)
    cb = np.asarray(inputs["conv_b"], np.float32)
    xpw = np.asarray(inputs["x_proj_w"], np.float32)
    dpw = np.asarray(inputs["dt_proj_w"], np.float32)
    dpb = np.asarray(inputs["dt_proj_b"], np.float32)
    alog = np.asarray(inputs["A_log"], np.float32)
    dv = np.asarray(inputs["D"], np.float32)
    wo = np.asarray(inputs["out_proj_w"], np.float32)

    hT = np.ascontiguousarray(hs.reshape(T, DM).T).astype(BF)
    woutT = np.ascontiguousarray(wo.T).reshape(2 * NCORES, 128, DM).astype(BF)
    ident = np.eye(128, dtype=np.float32).astype(BF)

    in_maps = []
    for i in range(NCORES):
        lo = i * DL
        sl = slice(lo, lo + DL)
        wxzT = np.ascontiguousarray(
            np.concatenate([wxz[sl], wxz[DI + lo:DI + lo + DL]], axis=0).T).astype(BF)
        cdiag = np.zeros((DC, NDH, 128, 128), np.float32)
        for j in range(DC):
            for h in range(NDH):
                np.fill_diagonal(cdiag[j, h], cw[lo + h * 128:lo + (h + 1) * 128, j])
        in_maps.append({
            "hT": hT,
            "wxzT": wxzT,
            "cdiag": cdiag.astype(BF),
            "convb": cb[sl].reshape(NDH, 128, 1),
            "xprojT": np.ascontiguousarray(xpw[:, sl].T).reshape(NDH, 128, 96).astype(BF),
            "dtwT": np.ascontiguousarray(dpw[sl].T).astype(BF),
            "dtb": dpb[sl].reshape(NDH, 128, 1),
            "aneg": (-np.exp(alog[sl])).reshape(NDH, 128, DS).astype(np.float32),
            "dvec": dv[sl].reshape(NDH, 128, 1),
            "woutT": woutT,
            "ident": ident,
        })
    return in_maps


def _run(inputs, trace=False, **kw):
    if "nc" not in _cached:
        _cached["nc"] = _build()
    nc = _cached["nc"]
    in_maps = _host_prep(inputs)
    res = bass_utils.run_bass_kernel_spmd(
        nc, in_maps, core_ids=list(range(NCORES)), trace=trace, **kw)
    # core j's out_slice rows: block k (128 rows) -> global t = k*1024 + j*128
    full = np.empty((NR, NCORES, 128, DM), np.float32)
    for j in range(NCORES):
        full[:, j] = res.results[j]["out_slice"].reshape(NR, 128, DM)
    return full.reshape(B, L, DM).astype(np.float32), res


def kernel(**inputs):
    out, _ = _run(inputs, trace=False)
    return out


# revision 24
# speedup vs baseline: 1.0000x; 1.0000x over previous
"""Trainium2 Bass kernel for a dense Mamba (selective-scan) block, SPMD over 8 NeuronCores.

Sharding: tensor-parallel over d_inner (2048 -> 256 channels/core).

v2: fully software-pipelined over 8 rounds of 1024 timesteps each:
  round k: AllReduce(k) | in_proj+conv+x_proj for round k+1 (PE/Scalar) |
           dt_proj+softplus (B) | selective scan (C, DVE-bound) |
           tiny AllToAll(k) | out_proj for t-block k (E, PE).
Output t-rows are interleaved in 128-blocks across cores so each round's
gated activations can be exchanged and out_proj'd immediately, hiding the
collective+out_proj tail under the scan. Activations (xs, silu(z), dts,
dtx) stay in SBUF rings; out_proj weights stay resident in SBUF.
DVE runs only the scan chain (exp on Scalar via act tables incl. native
Silu; PSUM->SBUF copies and casts on Scalar; gate epilogue fused via
scalar_tensor_tensor reading PSUM directly).

Shapes hardcoded for: B=2, L=4096, d_model=1024, d_inner=2048, d_state=16,
d_conv=4, dt_rank=64, f32 I/O.
"""
import numpy as np
import ml_dtypes
from contextlib import ExitStack

import concourse.bass as bass
import concourse.bacc as bacc
import concourse.tile as tile
from concourse import mybir
from concourse import bass_utils

BF = ml_dtypes.bfloat16
F32 = mybir.dt.float32
BF16 = mybir.dt.bfloat16

NCORES = 8
B, L, DM = 2, 4096, 1024
DI, DS, DC, DTR = 2048, 16, 4, 64
DL = DI // NCORES          # 256 local channels
NDH = DL // 128            # 2 d-half tiles
T = B * L                  # 8192 flattened (b, l)
TCB = 1024                 # round granularity (t per round)
NR = T // TCB              # 8 rounds
TCA = 512                  # in_proj chunk
ACT = mybir.ActivationFunctionType
ALU = mybir.AluOpType

_cached = {}


def _build():
    nc = bacc.Bacc("TRN2", target_bir_lowering=False, num_devices=NCORES)

    # ---- I/O -------------------------------------------------------------
    d_hT = nc.dram_tensor("hT", (DM, T), BF16, kind="ExternalInput")
    d_wxzT = nc.dram_tensor("wxzT", (DM, 2 * DL), BF16, kind="ExternalInput")
    d_cdiag = nc.dram_tensor("cdiag", (DC, NDH, 128, 128), BF16, kind="ExternalInput")
    d_convb = nc.dram_tensor("convb", (NDH, 128, 1), F32, kind="ExternalInput")
    d_xprojT = nc.dram_tensor("xprojT", (NDH, 128, DTR + 2 * DS), BF16, kind="ExternalInput")
    d_dtwT = nc.dram_tensor("dtwT", (DTR, DL), BF16, kind="ExternalInput")
    d_dtb = nc.dram_tensor("dtb", (NDH, 128, 1), F32, kind="ExternalInput")
    d_aneg = nc.dram_tensor("aneg", (NDH, 128, DS), F32, kind="ExternalInput")
    d_dvec = nc.dram_tensor("dvec", (NDH, 128, 1), F32, kind="ExternalInput")
    d_woutT = nc.dram_tensor("woutT", (2 * NCORES, 128, DM), BF16, kind="ExternalInput")
    d_ident = nc.dram_tensor("ident", (128, 128), BF16, kind="ExternalInput")
    d_out = nc.dram_tensor("out_slice", (NR * 128, DM), F32, kind="ExternalOutput")

    # ---- internal DRAM ---------------------------------------------------
    d_xdp = nc.dram_tensor("xdp", (2 * NR, DTR + 2 * DS, 512), F32, kind="Internal")
    d_xd = nc.dram_tensor("xd", (2 * NR, DTR + 2 * DS, 512), F32, kind="Internal",
                          addr_space="Shared")
    d_bc = nc.dram_tensor("bcrows", (2 * DS, T), BF16, kind="Internal")
    # tiny warm-up collective to absorb initial inter-core skew during A(0)
    d_ri = nc.dram_tensor("warm_i", (4, 64), F32, kind="Internal")
    d_ro = nc.dram_tensor("warm_o", (4, 64), F32, kind="Internal",
                          addr_space="Shared")
    # per-round A2A pieces: [round][target core][local ch][128 t]
    d_a2ai = nc.dram_tensor("a2ai", (NR, NCORES, DL, 128), BF16, kind="Internal")
    d_a2ao = nc.dram_tensor("a2ao", (NR, NCORES, DL, 128), BF16, kind="Internal")

    groups = [list(range(NCORES))]

    with tile.TileContext(nc) as tc, ExitStack() as ctx:
        consts = ctx.enter_context(tc.tile_pool(name="consts", bufs=1))
        arena = ctx.enter_context(tc.tile_pool(name="arena", bufs=1))
        work = ctx.enter_context(tc.tile_pool(name="work", bufs=2))
        psA = ctx.enter_context(tc.tile_pool(name="psA", bufs=2, space="PSUM"))
        psY = ctx.enter_context(tc.tile_pool(name="psY", bufs=4, space="PSUM"))
        psE = ctx.enter_context(tc.tile_pool(name="psE", bufs=2, space="PSUM"))

        # ---- constants ---------------------------------------------------
        wxz = consts.tile([128, 8, 2 * DL], BF16, tag="wxz")
        nc.sync.dma_start(out=wxz, in_=d_wxzT[:, :].rearrange("(k p) m -> p k m", p=128))
        cdg = consts.tile([128, DC, NDH, 128], BF16, tag="cdg")
        nc.sync.dma_start(
            out=cdg, in_=bass.AP(tensor=d_cdiag[:, :, :, :].tensor, offset=0,
                                 ap=[[128, 128], [NDH * 128 * 128, DC], [128 * 128, NDH], [1, 128]]))
        convb = consts.tile([128, NDH, 1], F32, tag="convb")
        nc.sync.dma_start(out=convb, in_=d_convb[:, :, :].rearrange("h p one -> p h one"))
        xprj = consts.tile([128, NDH, DTR + 2 * DS], BF16, tag="xprj")
        nc.sync.dma_start(out=xprj, in_=d_xprojT[:, :, :].rearrange("h p m -> p h m"))
        dtw = consts.tile([DTR, DL], BF16, tag="dtw")
        nc.sync.dma_start(out=dtw, in_=d_dtwT[:, :])
        dtb = consts.tile([128, NDH, 1], F32, tag="dtb")
        nc.sync.dma_start(out=dtb, in_=d_dtb[:, :, :].rearrange("h p one -> p h one"))
        aneg = consts.tile([128, NDH, DS], F32, tag="aneg")
        nc.sync.dma_start(out=aneg, in_=d_aneg[:, :, :].rearrange("h p n -> p h n"))
        dvec = consts.tile([128, NDH, 1], F32, tag="dvec")
        nc.sync.dma_start(out=dvec, in_=d_dvec[:, :, :].rearrange("h p one -> p h one"))
        ident = consts.tile([128, 128], BF16, tag="ident")
        nc.sync.dma_start(out=ident, in_=d_ident[:, :])
        wout = consts.tile([128, 2 * NCORES, DM], BF16, tag="wout")
        nc.sync.dma_start(out=wout, in_=d_woutT[:, :, :].rearrange("k p m -> p k m"))
        carry = consts.tile([128, NDH, DS], F32, tag="carry")

        # ---- SBUF activation rings --------------------------------------
        xpad = arena.tile([128, NDH, B, 3 + L], BF16, tag="xpad")
        xs_r = arena.tile([128, NDH, 3, TCB], BF16, tag="xs_r")     # ring-3 by round
        zs_r = arena.tile([128, NDH, 3, TCB], BF16, tag="zs_r")     # ring-3 (silu(z))
        dts_r = arena.tile([128, NDH, 2, TCB], BF16, tag="dts_r")   # ring-2
        dtx_r = arena.tile([128, NDH, 2, TCB], BF16, tag="dtx_r")   # ring-2

        for h in range(NDH):
            for b in range(B):
                nc.vector.memset(xpad[:, h, b, 0:3], 0.0)

        # ---- phase bodies ------------------------------------------------
        def phase_A(k):
            """in_proj + z-silu + conv-silu + x_proj partial for round k."""
            b, l0 = (k * TCB) // L, (k * TCB) % L
            r3, r2 = k % 3, k % 2
            for ci in range(TCB // TCA):  # 2 chunks of 512
                t0 = k * TCB + ci * TCA
                lc = l0 + ci * TCA
                ht = work.tile([128, 8, TCA], BF16, tag="ht")
                nc.sync.dma_start(
                    out=ht,
                    in_=bass.AP(tensor=d_hT[:, :].tensor, offset=t0,
                                ap=[[T, 128], [128 * T, 8], [1, TCA]]))
                for m in range(4):  # 0,1: x halves; 2,3: z halves
                    pxz = psA.tile([128, TCA], F32, tag="ps", name=f"pxz_{k}_{ci}_{m}")
                    for kk in range(8):
                        nc.tensor.matmul(pxz, lhsT=wxz[:, kk, m * 128:(m + 1) * 128],
                                         rhs=ht[:, kk, :], start=(kk == 0), stop=(kk == 7))
                    if m < 2:
                        nc.scalar.copy(xpad[:, m, b, 3 + lc: 3 + lc + TCA], pxz)
                    else:
                        nc.scalar.activation(
                            zs_r[:, m - 2, r3, ci * TCA:(ci + 1) * TCA], pxz, ACT.Silu)
                # conv for this chunk (xpad for it was just written)
                for h in range(NDH):
                    pc = psA.tile([128, TCA], F32, tag="ps", name=f"pc_{k}_{ci}_{h}")
                    for j in range(DC):
                        nc.tensor.matmul(pc, lhsT=cdg[:, j, h, :],
                                         rhs=xpad[:, h, b, lc + j: lc + j + TCA],
                                         start=(j == 0), stop=(j == DC - 1))
                    nc.scalar.activation(
                        xs_r[:, h, r3, ci * TCA:(ci + 1) * TCA], pc, ACT.Silu,
                        bias=convb[:, h, 0:1], scale=1.0)
                # x_proj partial for this chunk
                pxp = psA.tile([96, TCA], F32, tag="ps", name=f"pxp_{k}_{ci}")
                for h in range(NDH):
                    nc.tensor.matmul(pxp, lhsT=xprj[:, h, :],
                                     rhs=xs_r[:, h, r3, ci * TCA:(ci + 1) * TCA],
                                     start=(h == 0), stop=(h == NDH - 1))
                xpt = work.tile([96, TCA], F32, tag="xpt")
                nc.scalar.copy(xpt, pxp)
                nc.sync.dma_start(out=d_xdp[2 * k + ci, :, :], in_=xpt)

        def phase_B(k, q0, q1):
            """dt_proj + softplus + dtx for column-halves [q0, q1) of round k."""
            t0 = k * TCB
            r3, r2 = k % 3, k % 2
            spes = []
            for qq in range(q0, q1):
                xdt = work.tile([96, 512], F32, tag="xdt", name=f"xdt_{k}_{qq}")
                nc.sync.dma_start(out=xdt, in_=d_xd[2 * k + qq, :, :])
                xdb = work.tile([96, 512], BF16, tag="xdb", name=f"xdb_{k}_{qq}")
                nc.scalar.copy(xdb, xdt)
                nc.sync.dma_start(out=d_bc[:, t0 + qq * 512:t0 + (qq + 1) * 512],
                                  in_=xdb[DTR:DTR + 2 * DS, :])
                for h in range(NDH):
                    pdt = psA.tile([128, 512], F32, tag="ps", name=f"pdt_{k}_{h}_{qq}")
                    nc.tensor.matmul(pdt, lhsT=dtw[:, h * 128:(h + 1) * 128],
                                     rhs=xdb[0:DTR, :], start=True, stop=True)
                    spe = work.tile([128, 512], F32, tag="spe", bufs=4,
                                    name=f"spe_{k}_{h}_{qq}")
                    nc.scalar.activation(spe, pdt, ACT.Exp,
                                         bias=dtb[:, h, 0:1], scale=1.0)
                    spes.append((h, qq, spe))
            for h, qq, spe in spes:
                nc.scalar.activation(dts_r[:, h, r2, qq * 512:(qq + 1) * 512],
                                     spe, ACT.Ln, bias=1.0, scale=1.0)
            for h in range(NDH):
                nc.vector.tensor_mul(dtx_r[:, h, r2, q0 * 512:q1 * 512],
                                     dts_r[:, h, r2, q0 * 512:q1 * 512],
                                     xs_r[:, h, r3, q0 * 512:q1 * 512])

        def phase_C(k, q0, q1, pys):
            """selective scan for columns [q0*512, q1*512) of round k."""
            c0, W = q0 * 512, (q1 - q0) * 512
            t0c = k * TCB + c0
            r3, r2 = k % 3, k % 2
            first_in_seq = (t0c % L == 0)
            copy_carry = ((t0c + W) % L != 0)
            for j in range(DS // 2):  # n-pairs
                bbc = work.tile([128, 2, W], BF16, tag="bbc",
                                name=f"bbc_{k}_{q0}_{j}")
                cbc = work.tile([128, 2, W], BF16, tag="cbc",
                                name=f"cbc_{k}_{q0}_{j}")
                for nn in range(2):
                    n = 2 * j + nn
                    nc.sync.dma_start(
                        out=bbc[:, nn, :],
                        in_=bass.AP(tensor=d_bc[:, :].tensor, offset=n * T + t0c,
                                    ap=[[0, 128], [1, W]]))
                    nc.sync.dma_start(
                        out=cbc[:, nn, :],
                        in_=bass.AP(tensor=d_bc[:, :].tensor, offset=(DS + n) * T + t0c,
                                    ap=[[0, 128], [1, W]]))
                for h in range(NDH):
                    dA = [None, None]
                    for nn in range(2):
                        n = 2 * j + nn
                        dA[nn] = work.tile([128, W], BF16, tag="dA", bufs=3,
                                           name=f"dA_{k}_{q0}_{j}_{h}_{nn}")
                        nc.scalar.activation(dA[nn], dts_r[:, h, r2, c0:c0 + W],
                                             ACT.Exp, bias=0.0,
                                             scale=aneg[:, h, n:n + 1])
                    dtxs = dtx_r[:, h, r2, c0:c0 + W]
                    dBx = work.tile([128, 2, W], BF16, tag="dBx", bufs=1,
                                    name=f"dBx_{k}_{q0}_{j}_{h}")
                    nc.vector.tensor_mul(
                        dBx,
                        bass.AP(tensor=dtxs.tensor, offset=dtxs.offset,
                                ap=[dtxs.ap[0], [0, 2], dtxs.ap[1]]),
                        bbc)
                    hts = work.tile([128, 2, W], BF16, tag="hts", bufs=1,
                                    name=f"hts_{k}_{q0}_{j}_{h}")
                    for nn in range(2):
                        n = 2 * j + nn
                        init = 0.0 if first_in_seq else carry[:, h, n:n + 1]
                        nc.vector.tensor_tensor_scan(
                            out=hts[:, nn, :], data0=dA[nn], data1=dBx[:, nn, :],
                            initial=init, op0=ALU.mult, op1=ALU.add)
                        if copy_carry:
                            nc.vector.tensor_copy(carry[:, h, n:n + 1],
                                                  hts[:, nn, W - 1:W])
                    yp = work.tile([128, 2, W], BF16, tag="yp", bufs=1,
                                   name=f"yp_{k}_{q0}_{j}_{h}")
                    nc.vector.tensor_mul(yp, hts, cbc)
                    for nn in range(2):
                        for qq in range(q0, q1):
                            nc.tensor.matmul(
                                pys[h][qq], lhsT=ident,
                                rhs=yp[:, nn, (qq - q0) * 512:(qq - q0 + 1) * 512],
                                start=(j == 0 and nn == 0),
                                stop=(j == DS // 2 - 1 and nn == 1))
            if q1 != 2:
                return
            # epilogue: ys = (xs*D + psY) * silu(z); scatter to A2A input
            for h in range(NDH):
                ys = work.tile([128, TCB], BF16, tag="ys", name=f"ys_{k}_{h}")
                for qq in range(2):
                    nc.scalar.copy(ys[:, qq * 512:(qq + 1) * 512], pys[h][qq])
                xsd = work.tile([128, TCB], BF16, tag="xsd", name=f"xsd_{k}_{h}")
                nc.vector.tensor_scalar_mul(xsd, xs_r[:, h, r3, :], dvec[:, h, 0:1])
                nc.vector.tensor_add(ys, ys, xsd)
                nc.vector.tensor_mul(ys, ys, zs_r[:, h, r3, :])
                for jsh in range(NCORES):
                    nc.sync.dma_start(
                        out=d_a2ai[k, jsh, h * 128:(h + 1) * 128, :],
                        in_=ys[:, jsh * 128:(jsh + 1) * 128])

        def phase_E(k):
            """out_proj for my interleaved t-block of round k."""
            yblk = work.tile([128, NCORES, NDH, 128], BF16, tag="yblk")
            for iq in range(4):  # split across DMA queues
                nc.sync.dma_start(
                    out=yblk[:, 2 * iq:2 * iq + 2, :, :],
                    in_=bass.AP(tensor=d_a2ao[:, :, :, :].tensor,
                                offset=(k * NCORES + 2 * iq) * DL * 128,
                                ap=[[128, 128], [DL * 128, 2], [128 * 128, NDH], [1, 128]]))
            for fh in range(2):
                pe = psE.tile([128, 512], F32, tag="pe", name=f"pe_{k}_{fh}")
                for kt in range(2 * NCORES):
                    i, h = kt // 2, kt % 2
                    nc.tensor.matmul(pe, lhsT=yblk[:, i, h, :],
                                     rhs=wout[:, kt, fh * 512:(fh + 1) * 512],
                                     start=(kt == 0), stop=(kt == 2 * NCORES - 1))
                ot = work.tile([128, 512], F32, tag="ot", bufs=1)
                nc.scalar.copy(ot, pe)
                nc.sync.dma_start(
                    out=d_out[k * 128:(k + 1) * 128, fh * 512:(fh + 1) * 512],
                    in_=ot)

        def all_reduce(i):
            nc.gpsimd.collective_compute(
                kind="AllReduce", op=ALU.add, replica_groups=groups,
                ins=[d_xdp[i, :, :]], outs=[d_xd[i, :, :]])

        def all_reduce_full(k):
            nc.gpsimd.collective_compute(
                kind="AllReduce", op=ALU.add, replica_groups=groups,
                ins=[d_xdp[2 * k:2 * k + 2, :, :]], outs=[d_xd[2 * k:2 * k + 2, :, :]])

        def all_to_all(k):
            nc.gpsimd.collective_compute(
                kind="AllToAll", op=ALU.bypass, replica_groups=groups,
                ins=[d_a2ai[k, :, :, :]], outs=[d_a2ao[k, :, :, :]])

        # ---- pipelined schedule -----------------------------------------
        warm = consts.tile([4, 64], F32, tag="warm")
        nc.vector.memset(warm, 0.0)
        nc.sync.dma_start(out=d_ri[:, :], in_=warm)
        nc.gpsimd.collective_compute(
            kind="AllReduce", op=ALU.add, replica_groups=groups,
            ins=[d_ri[:, :]], outs=[d_ro[:, :]])
        phase_A(0)
        all_reduce(0)
        all_reduce(1)
        for k in range(NR):
            pys = [[psY.tile([128, 512], F32, tag="py", name=f"pys_{k}_{h}_{qq}")
                    for qq in range(2)] for h in range(NDH)]
            if k == 0:
                # round 0: run B/C per half-round to start the scan ASAP,
                # before enqueueing the next round's A work
                phase_B(0, 0, 1)
                phase_C(0, 0, 1, pys)
                phase_A(1)
                all_reduce_full(1)
                phase_B(0, 1, 2)
                phase_C(0, 1, 2, pys)
            else:
                if k + 1 < NR:
                    phase_A(k + 1)
                    all_reduce_full(k + 1)
                phase_B(k, 0, 2)
                phase_C(k, 0, 2, pys)
            all_to_all(k)
            phase_E(k)

    nc.compile()
    return nc


def _host_prep(inputs):
    """Per-core input maps from full inputs (layout prep + bf16 casts only)."""
    hs = np.asarray(inputs["hidden_states"], np.float32)
    wxz = np.asarray(inputs["in_proj_w"], np.float32)
    cw = np.asarray(inputs["conv_w"], np.float32)
    cb = np.asarray(inputs["conv_b"], np.float32)
    xpw = np.asarray(inputs["x_proj_w"], np.float32)
    dpw = np.asarray(inputs["dt_proj_w"], np.float32)
    dpb = np.asarray(inputs["dt_proj_b"], np.float32)
    alog = np.asarray(inputs["A_log"], np.float32)
    dv = np.asarray(inputs["D"], np.float32)
    wo = np.asarray(inputs["out_proj_w"], np.float32)

    hT = np.ascontiguousarray(hs.reshape(T, DM).T).astype(BF)
    woutT = np.ascontiguousarray(wo.T).reshape(2 * NCORES, 128, DM).astype(BF)
    ident = np.eye(128, dtype=np.float32).astype(BF)

    in_maps = []
    for i in range(NCORES):
        lo = i * DL
        sl = slice(lo, lo + DL)
        wxzT = np.ascontiguousarray(
            np.concatenate([wxz[sl], wxz[DI + lo:DI + lo + DL]], axis=0).T).astype(BF)
        cdiag = np.zeros((DC, NDH, 128, 128), np.float32)
        for j in range(DC):
            for h in range(NDH):
                np.fill_diagonal(cdiag[j, h], cw[lo + h * 128:lo + (h + 1) * 128, j])
        in_maps.append({
            "hT": hT,
            "wxzT": wxzT,
            "cdiag": cdiag.astype(BF),
            "convb": cb[sl].reshape(NDH, 128, 1),
            "xprojT": np.ascontiguousarray(xpw[:, sl].T).reshape(NDH, 128, 96).astype(BF),
            "dtwT": np.ascontiguousarray(dpw[sl].T).astype(BF),
            "dtb": dpb[sl].reshape(NDH, 128, 1),
            "aneg": (-np.exp(alog[sl])).reshape(NDH, 128, DS).astype(np.float32),
            "dvec": dv[sl].reshape(NDH, 128, 1),
            "woutT": woutT,
            "ident": ident,
        })
    return in_maps


def _run(inputs, trace=False, **kw):
    if "nc" not in _cached:
        _cached["nc"] = _build()
    nc = _cached["nc"]
    in_maps = _host_prep(inputs)
    res = bass_utils.run_bass_kernel_spmd(
        nc, in_maps, core_ids=list(range(NCORES)), trace=trace, **kw)
    # core j's out_slice rows: block k (128 rows) -> global t = k*1024 + j*128
    full = np.empty((NR, NCORES, 128, DM), np.float32)
    for j in range(NCORES):
        full[:, j] = res.results[j]["out_slice"].reshape(NR, 128, DM)
    return full.reshape(B, L, DM).astype(np.float32), res


def kernel(**inputs):
    out, _ = _run(inputs, trace=False)
    return out


# revision 25
# speedup vs baseline: 1.0047x; 1.0047x over previous
"""Trainium2 Bass kernel for a dense Mamba (selective-scan) block, SPMD over 8 NeuronCores.

Sharding: tensor-parallel over d_inner (2048 -> 256 channels/core).

v2: fully software-pipelined over 8 rounds of 1024 timesteps each:
  round k: AllReduce(k) | in_proj+conv+x_proj for round k+1 (PE/Scalar) |
           dt_proj+softplus (B) | selective scan (C, DVE-bound) |
           tiny AllToAll(k) | out_proj for t-block k (E, PE).
Output t-rows are interleaved in 128-blocks across cores so each round's
gated activations can be exchanged and out_proj'd immediately, hiding the
collective+out_proj tail under the scan. Activations (xs, silu(z), dts,
dtx) stay in SBUF rings; out_proj weights stay resident in SBUF.
DVE runs only the scan chain (exp on Scalar via act tables incl. native
Silu; PSUM->SBUF copies and casts on Scalar; gate epilogue fused via
scalar_tensor_tensor reading PSUM directly).

Shapes hardcoded for: B=2, L=4096, d_model=1024, d_inner=2048, d_state=16,
d_conv=4, dt_rank=64, f32 I/O.
"""
import numpy as np
import ml_dtypes
from contextlib import ExitStack

import concourse.bass as bass
import concourse.bacc as bacc
import concourse.tile as tile
from concourse import mybir
from concourse import bass_utils

BF = ml_dtypes.bfloat16
F32 = mybir.dt.float32
BF16 = mybir.dt.bfloat16

NCORES = 8
B, L, DM = 2, 4096, 1024
DI, DS, DC, DTR = 2048, 16, 4, 64
DL = DI // NCORES          # 256 local channels
NDH = DL // 128            # 2 d-half tiles
T = B * L                  # 8192 flattened (b, l)
TCB = 1024                 # round granularity (t per round)
NR = T // TCB              # 8 rounds
TCA = 512                  # in_proj chunk
ACT = mybir.ActivationFunctionType
ALU = mybir.AluOpType

_cached = {}


def _build():
    nc = bacc.Bacc("TRN2", target_bir_lowering=False, num_devices=NCORES)

    # ---- I/O -------------------------------------------------------------
    d_hT = nc.dram_tensor("hT", (DM, T), BF16, kind="ExternalInput")
    d_wxzT = nc.dram_tensor("wxzT", (DM, 2 * DL), BF16, kind="ExternalInput")
    d_cdiag = nc.dram_tensor("cdiag", (DC, NDH, 128, 128), BF16, kind="ExternalInput")
    d_convb = nc.dram_tensor("convb", (NDH, 128, 1), F32, kind="ExternalInput")
    d_xprojT = nc.dram_tensor("xprojT", (NDH, 128, DTR + 2 * DS), BF16, kind="ExternalInput")
    d_dtwT = nc.dram_tensor("dtwT", (DTR, DL), BF16, kind="ExternalInput")
    d_dtb = nc.dram_tensor("dtb", (NDH, 128, 1), F32, kind="ExternalInput")
    d_aneg = nc.dram_tensor("aneg", (NDH, 128, DS), F32, kind="ExternalInput")
    d_dvec = nc.dram_tensor("dvec", (NDH, 128, 1), F32, kind="ExternalInput")
    d_woutT = nc.dram_tensor("woutT", (2 * NCORES, 128, DM), BF16, kind="ExternalInput")
    d_ident = nc.dram_tensor("ident", (128, 128), BF16, kind="ExternalInput")
    d_out = nc.dram_tensor("out_slice", (NR * 128, DM), F32, kind="ExternalOutput")

    # ---- internal DRAM ---------------------------------------------------
    d_xdp = nc.dram_tensor("xdp", (2 * NR, DTR + 2 * DS, 512), F32, kind="Internal")
    d_xd = nc.dram_tensor("xd", (2 * NR, DTR + 2 * DS, 512), F32, kind="Internal",
                          addr_space="Shared")
    d_bc = nc.dram_tensor("bcrows", (2 * DS, T), BF16, kind="Internal")
    # tiny warm-up collective to absorb initial inter-core skew during A(0)
    d_ri = nc.dram_tensor("warm_i", (4, 64), F32, kind="Internal")
    d_ro = nc.dram_tensor("warm_o", (4, 64), F32, kind="Internal",
                          addr_space="Shared")
    # per-round A2A pieces: [round][target core][local ch][128 t]
    d_a2ai = nc.dram_tensor("a2ai", (NR, NCORES, DL, 128), BF16, kind="Internal")
    d_a2ao = nc.dram_tensor("a2ao", (NR, NCORES, DL, 128), BF16, kind="Internal")

    groups = [list(range(NCORES))]

    with tile.TileContext(nc) as tc, ExitStack() as ctx:
        consts = ctx.enter_context(tc.tile_pool(name="consts", bufs=1))
        arena = ctx.enter_context(tc.tile_pool(name="arena", bufs=1))
        work = ctx.enter_context(tc.tile_pool(name="work", bufs=2))
        psA = ctx.enter_context(tc.tile_pool(name="psA", bufs=2, space="PSUM"))
        psY = ctx.enter_context(tc.tile_pool(name="psY", bufs=4, space="PSUM"))
        psE = ctx.enter_context(tc.tile_pool(name="psE", bufs=2, space="PSUM"))

        # ---- constants ---------------------------------------------------
        wxz = consts.tile([128, 8, 2 * DL], BF16, tag="wxz")
        nc.sync.dma_start(out=wxz, in_=d_wxzT[:, :].rearrange("(k p) m -> p k m", p=128))
        cdg = consts.tile([128, DC, NDH, 128], BF16, tag="cdg")
        nc.sync.dma_start(
            out=cdg, in_=bass.AP(tensor=d_cdiag[:, :, :, :].tensor, offset=0,
                                 ap=[[128, 128], [NDH * 128 * 128, DC], [128 * 128, NDH], [1, 128]]))
        convb = consts.tile([128, NDH, 1], F32, tag="convb")
        nc.sync.dma_start(out=convb, in_=d_convb[:, :, :].rearrange("h p one -> p h one"))
        xprj = consts.tile([128, NDH, DTR + 2 * DS], BF16, tag="xprj")
        nc.sync.dma_start(out=xprj, in_=d_xprojT[:, :, :].rearrange("h p m -> p h m"))
        dtw = consts.tile([DTR, DL], BF16, tag="dtw")
        nc.sync.dma_start(out=dtw, in_=d_dtwT[:, :])
        dtb = consts.tile([128, NDH, 1], F32, tag="dtb")
        nc.sync.dma_start(out=dtb, in_=d_dtb[:, :, :].rearrange("h p one -> p h one"))
        aneg = consts.tile([128, NDH, DS], F32, tag="aneg")
        nc.sync.dma_start(out=aneg, in_=d_aneg[:, :, :].rearrange("h p n -> p h n"))
        dvec = consts.tile([128, NDH, 1], F32, tag="dvec")
        nc.sync.dma_start(out=dvec, in_=d_dvec[:, :, :].rearrange("h p one -> p h one"))
        ident = consts.tile([128, 128], BF16, tag="ident")
        nc.sync.dma_start(out=ident, in_=d_ident[:, :])
        wout = consts.tile([128, 2 * NCORES, DM], BF16, tag="wout")
        nc.sync.dma_start(out=wout, in_=d_woutT[:, :, :].rearrange("k p m -> p k m"))
        carry = consts.tile([128, NDH, DS], F32, tag="carry")

        # ---- SBUF activation rings --------------------------------------
        xpad = arena.tile([128, NDH, B, 3 + L], BF16, tag="xpad")
        xs_r = arena.tile([128, NDH, 3, TCB], BF16, tag="xs_r")     # ring-3 by round
        zs_r = arena.tile([128, NDH, 3, TCB], BF16, tag="zs_r")     # ring-3 (silu(z))
        dts_r = arena.tile([128, NDH, 2, TCB], BF16, tag="dts_r")   # ring-2
        dtx_r = arena.tile([128, NDH, 2, TCB], BF16, tag="dtx_r")   # ring-2

        for h in range(NDH):
            for b in range(B):
                nc.vector.memset(xpad[:, h, b, 0:3], 0.0)

        # ---- phase bodies ------------------------------------------------
        def phase_A(k):
            """in_proj + z-silu + conv-silu + x_proj partial for round k."""
            b, l0 = (k * TCB) // L, (k * TCB) % L
            r3, r2 = k % 3, k % 2
            for ci in range(TCB // TCA):  # 2 chunks of 512
                t0 = k * TCB + ci * TCA
                lc = l0 + ci * TCA
                ht = work.tile([128, 8, TCA], BF16, tag="ht")
                nc.sync.dma_start(
                    out=ht,
                    in_=bass.AP(tensor=d_hT[:, :].tensor, offset=t0,
                                ap=[[T, 128], [128 * T, 8], [1, TCA]]))
                for m in range(4):  # 0,1: x halves; 2,3: z halves
                    pxz = psA.tile([128, TCA], F32, tag="ps", name=f"pxz_{k}_{ci}_{m}")
                    for kk in range(8):
                        nc.tensor.matmul(pxz, lhsT=wxz[:, kk, m * 128:(m + 1) * 128],
                                         rhs=ht[:, kk, :], start=(kk == 0), stop=(kk == 7))
                    if m < 2:
                        nc.scalar.copy(xpad[:, m, b, 3 + lc: 3 + lc + TCA], pxz)
                    else:
                        nc.scalar.activation(
                            zs_r[:, m - 2, r3, ci * TCA:(ci + 1) * TCA], pxz, ACT.Silu)
                # conv for this chunk (xpad for it was just written)
                for h in range(NDH):
                    pc = psA.tile([128, TCA], F32, tag="ps", name=f"pc_{k}_{ci}_{h}")
                    for j in range(DC):
                        nc.tensor.matmul(pc, lhsT=cdg[:, j, h, :],
                                         rhs=xpad[:, h, b, lc + j: lc + j + TCA],
                                         start=(j == 0), stop=(j == DC - 1))
                    nc.scalar.activation(
                        xs_r[:, h, r3, ci * TCA:(ci + 1) * TCA], pc, ACT.Silu,
                        bias=convb[:, h, 0:1], scale=1.0)
                # x_proj partial for this chunk
                pxp = psA.tile([96, TCA], F32, tag="ps", name=f"pxp_{k}_{ci}")
                for h in range(NDH):
                    nc.tensor.matmul(pxp, lhsT=xprj[:, h, :],
                                     rhs=xs_r[:, h, r3, ci * TCA:(ci + 1) * TCA],
                                     start=(h == 0), stop=(h == NDH - 1))
                xpt = work.tile([96, TCA], F32, tag="xpt")
                nc.scalar.copy(xpt, pxp)
                nc.sync.dma_start(out=d_xdp[2 * k + ci, :, :], in_=xpt)

        def phase_B(k, q0, q1):
            """dt_proj + softplus + dtx for column-halves [q0, q1) of round k."""
            t0 = k * TCB
            r3, r2 = k % 3, k % 2
            spes = []
            for qq in range(q0, q1):
                xdt = work.tile([96, 512], F32, tag="xdt", name=f"xdt_{k}_{qq}")
                nc.sync.dma_start(out=xdt, in_=d_xd[2 * k + qq, :, :])
                xdb = work.tile([96, 512], BF16, tag="xdb", name=f"xdb_{k}_{qq}")
                nc.scalar.copy(xdb, xdt)
                nc.sync.dma_start(out=d_bc[:, t0 + qq * 512:t0 + (qq + 1) * 512],
                                  in_=xdb[DTR:DTR + 2 * DS, :])
                for h in range(NDH):
                    pdt = psA.tile([128, 512], F32, tag="ps", name=f"pdt_{k}_{h}_{qq}")
                    nc.tensor.matmul(pdt, lhsT=dtw[:, h * 128:(h + 1) * 128],
                                     rhs=xdb[0:DTR, :], start=True, stop=True)
                    spe = work.tile([128, 512], F32, tag="spe", bufs=4,
                                    name=f"spe_{k}_{h}_{qq}")
                    nc.scalar.activation(spe, pdt, ACT.Exp,
                                         bias=dtb[:, h, 0:1], scale=1.0)
                    spes.append((h, qq, spe))
            for h, qq, spe in spes:
                nc.scalar.activation(dts_r[:, h, r2, qq * 512:(qq + 1) * 512],
                                     spe, ACT.Ln, bias=1.0, scale=1.0)
            for h in range(NDH):
                nc.vector.tensor_mul(dtx_r[:, h, r2, q0 * 512:q1 * 512],
                                     dts_r[:, h, r2, q0 * 512:q1 * 512],
                                     xs_r[:, h, r3, q0 * 512:q1 * 512])

        def phase_C(k, q0, q1, pys):
            """selective scan for columns [q0*512, q1*512) of round k."""
            c0, W = q0 * 512, (q1 - q0) * 512
            t0c = k * TCB + c0
            r3, r2 = k % 3, k % 2
            first_in_seq = (t0c % L == 0)
            copy_carry = ((t0c + W) % L != 0)
            for j in range(DS // 2):  # n-pairs
                bbc = work.tile([128, 2, W], BF16, tag="bbc",
                                name=f"bbc_{k}_{q0}_{j}")
                cbc = work.tile([128, 2, W], BF16, tag="cbc",
                                name=f"cbc_{k}_{q0}_{j}")
                for nn in range(2):
                    n = 2 * j + nn
                    nc.sync.dma_start(
                        out=bbc[:, nn, :],
                        in_=bass.AP(tensor=d_bc[:, :].tensor, offset=n * T + t0c,
                                    ap=[[0, 128], [1, W]]))
                    nc.sync.dma_start(
                        out=cbc[:, nn, :],
                        in_=bass.AP(tensor=d_bc[:, :].tensor, offset=(DS + n) * T + t0c,
                                    ap=[[0, 128], [1, W]]))
                for h in range(NDH):
                    dA = [None, None]
                    for nn in range(2):
                        n = 2 * j + nn
                        dA[nn] = work.tile([128, W], BF16, tag="dA", bufs=3,
                                           name=f"dA_{k}_{q0}_{j}_{h}_{nn}")
                        nc.scalar.activation(dA[nn], dts_r[:, h, r2, c0:c0 + W],
                                             ACT.Exp, bias=0.0,
                                             scale=aneg[:, h, n:n + 1])
                    dtxs = dtx_r[:, h, r2, c0:c0 + W]
                    dBx = work.tile([128, 2, W], BF16, tag="dBx", bufs=1,
                                    name=f"dBx_{k}_{q0}_{j}_{h}")
                    nc.vector.tensor_mul(
                        dBx,
                        bass.AP(tensor=dtxs.tensor, offset=dtxs.offset,
                                ap=[dtxs.ap[0], [0, 2], dtxs.ap[1]]),
                        bbc)
                    hts = work.tile([128, 2, W], BF16, tag="hts", bufs=1,
                                    name=f"hts_{k}_{q0}_{j}_{h}")
                    for nn in range(2):
                        n = 2 * j + nn
                        init = 0.0 if first_in_seq else carry[:, h, n:n + 1]
                        nc.vector.tensor_tensor_scan(
                            out=hts[:, nn, :], data0=dA[nn], data1=dBx[:, nn, :],
                            initial=init, op0=ALU.mult, op1=ALU.add)
                        if copy_carry:
                            nc.vector.tensor_copy(carry[:, h, n:n + 1],
                                                  hts[:, nn, W - 1:W])
                    yp = work.tile([128, 2, W], BF16, tag="yp", bufs=1,
                                   name=f"yp_{k}_{q0}_{j}_{h}")
                    nc.vector.tensor_mul(yp, hts, cbc)
                    for nn in range(2):
                        for qq in range(q0, q1):
                            nc.tensor.matmul(
                                pys[h][qq], lhsT=ident,
                                rhs=yp[:, nn, (qq - q0) * 512:(qq - q0 + 1) * 512],
                                start=(j == 0 and nn == 0),
                                stop=(j == DS // 2 - 1 and nn == 1))
            if q1 != 2:
                return
            # epilogue: ys = (xs*D + psY) * silu(z); scatter to A2A input
            # (pure-DVE: reading psY via scalar_tensor_tensor avoids a
            # Scalar-engine dependency stalling the DVE at round boundaries)
            for h in range(NDH):
                ys = work.tile([128, TCB], BF16, tag="ys", name=f"ys_{k}_{h}")
                for qq in range(2):
                    nc.vector.scalar_tensor_tensor(
                        out=ys[:, qq * 512:(qq + 1) * 512],
                        in0=xs_r[:, h, r3, qq * 512:(qq + 1) * 512],
                        scalar=dvec[:, h, 0:1],
                        in1=pys[h][qq], op0=ALU.mult, op1=ALU.add)
                nc.vector.tensor_mul(ys, ys, zs_r[:, h, r3, :])
                for jsh in range(NCORES):
                    nc.sync.dma_start(
                        out=d_a2ai[k, jsh, h * 128:(h + 1) * 128, :],
                        in_=ys[:, jsh * 128:(jsh + 1) * 128])

        def phase_E(k):
            """out_proj for my interleaved t-block of round k."""
            yblk = work.tile([128, NCORES, NDH, 128], BF16, tag="yblk")
            for iq in range(4):  # split across DMA queues
                nc.sync.dma_start(
                    out=yblk[:, 2 * iq:2 * iq + 2, :, :],
                    in_=bass.AP(tensor=d_a2ao[:, :, :, :].tensor,
                                offset=(k * NCORES + 2 * iq) * DL * 128,
                                ap=[[128, 128], [DL * 128, 2], [128 * 128, NDH], [1, 128]]))
            for fh in range(2):
                pe = psE.tile([128, 512], F32, tag="pe", name=f"pe_{k}_{fh}")
                for kt in range(2 * NCORES):
                    i, h = kt // 2, kt % 2
                    nc.tensor.matmul(pe, lhsT=yblk[:, i, h, :],
                                     rhs=wout[:, kt, fh * 512:(fh + 1) * 512],
                                     start=(kt == 0), stop=(kt == 2 * NCORES - 1))
                ot = work.tile([128, 512], F32, tag="ot", bufs=1)
                nc.scalar.copy(ot, pe)
                nc.sync.dma_start(
                    out=d_out[k * 128:(k + 1) * 128, fh * 512:(fh + 1) * 512],
                    in_=ot)

        def all_reduce(i):
            nc.gpsimd.collective_compute(
                kind="AllReduce", op=ALU.add, replica_groups=groups,
                ins=[d_xdp[i, :, :]], outs=[d_xd[i, :, :]])

        def all_reduce_full(k):
            nc.gpsimd.collective_compute(
                kind="AllReduce", op=ALU.add, replica_groups=groups,
                ins=[d_xdp[2 * k:2 * k + 2, :, :]], outs=[d_xd[2 * k:2 * k + 2, :, :]])

        def all_to_all(k):
            nc.gpsimd.collective_compute(
                kind="AllToAll", op=ALU.bypass, replica_groups=groups,
                ins=[d_a2ai[k, :, :, :]], outs=[d_a2ao[k, :, :, :]])

        # ---- pipelined schedule -----------------------------------------
        warm = consts.tile([4, 64], F32, tag="warm")
        nc.vector.memset(warm, 0.0)
        nc.sync.dma_start(out=d_ri[:, :], in_=warm)
        nc.gpsimd.collective_compute(
            kind="AllReduce", op=ALU.add, replica_groups=groups,
            ins=[d_ri[:, :]], outs=[d_ro[:, :]])
        phase_A(0)
        all_reduce(0)
        all_reduce(1)
        for k in range(NR):
            pys = [[psY.tile([128, 512], F32, tag="py", name=f"pys_{k}_{h}_{qq}")
                    for qq in range(2)] for h in range(NDH)]
            if k == 0:
                # round 0: run B/C per half-round to start the scan ASAP,
                # before enqueueing the next round's A work
                phase_B(0, 0, 1)
                phase_C(0, 0, 1, pys)
                phase_A(1)
                all_reduce_full(1)
                phase_B(0, 1, 2)
                phase_C(0, 1, 2, pys)
            else:
                if k + 1 < NR:
                    phase_A(k + 1)
                    all_reduce_full(k + 1)
                phase_B(k, 0, 2)
                phase_C(k, 0, 2, pys)
            all_to_all(k)
            phase_E(k)

    nc.compile()
    return nc


def _host_prep(inputs):
    """Per-core input maps from full inputs (layout prep + bf16 casts only)."""
    hs = np.asarray(inputs["hidden_states"], np.float32)
    wxz = np.asarray(inputs["in_proj_w"], np.float32)
    cw = np.asarray(inputs["conv_w"], np.float32)
    cb = np.asarray(inputs["conv_b"], np.float32)
    xpw = np.asarray(inputs["x_proj_w"], np.float32)
    dpw = np.asarray(inputs["dt_proj_w"], np.float32)
    dpb = np.asarray(inputs["dt_proj_b"], np.float32)
    alog = np.asarray(inputs["A_log"], np.float32)
    dv = np.asarray(inputs["D"], np.float32)
    wo = np.asarray(inputs["out_proj_w"], np.float32)

    hT = np.ascontiguousarray(hs.reshape(T, DM).T).astype(BF)
    woutT = np.ascontiguousarray(wo.T).reshape(2 * NCORES, 128, DM).astype(BF)
    ident = np.eye(128, dtype=np.float32).astype(BF)

    in_maps = []
    for i in range(NCORES):
        lo = i * DL
        sl = slice(lo, lo + DL)
        wxzT = np.ascontiguousarray(
            np.concatenate([wxz[sl], wxz[DI + lo:DI + lo + DL]], axis=0).T).astype(BF)
        cdiag = np.zeros((DC, NDH, 128, 128), np.float32)
        for j in range(DC):
            for h in range(NDH):
                np.fill_diagonal(cdiag[j, h], cw[lo + h * 128:lo + (h + 1) * 128, j])
        in_maps.append({
            "hT": hT,
            "wxzT": wxzT,
            "cdiag": cdiag.astype(BF),
            "convb": cb[sl].reshape(NDH, 128, 1),
            "xprojT": np.ascontiguousarray(xpw[:, sl].T).reshape(NDH, 128, 96).astype(BF),
            "dtwT": np.ascontiguousarray(dpw[sl].T).astype(BF),
            "dtb": dpb[sl].reshape(NDH, 128, 1),
            "aneg": (-np.exp(alog[sl])).reshape(NDH, 128, DS).astype(np.float32),
            "dvec": dv[sl].reshape(NDH, 128, 1),
            "woutT": woutT,
            "ident": ident,
        })
    return in_maps


def _run(inputs, trace=False, **kw):
    if "nc" not in _cached:
        _cached["nc"] = _build()
    nc = _cached["nc"]
    in_maps = _host_prep(inputs)
    res = bass_utils.run_bass_kernel_spmd(
        nc, in_maps, core_ids=list(range(NCORES)), trace=trace, **kw)
    # core j's out_slice rows: block k (128 rows) -> global t = k*1024 + j*128
    full = np.empty((NR, NCORES, 128, DM), np.float32)
    for j in range(NCORES):
        full[:, j] = res.results[j]["out_slice"].reshape(NR, 128, DM)
    return full.reshape(B, L, DM).astype(np.float32), res


def kernel(**inputs):
    out, _ = _run(inputs, trace=False)
    return out


# revision 32
# speedup vs baseline: 1.0145x; 1.0097x over previous
"""Trainium2 Bass kernel for a dense Mamba (selective-scan) block, SPMD over 8 NeuronCores.

Sharding: tensor-parallel over d_inner (2048 -> 256 channels/core).

v2: fully software-pipelined over 8 rounds of 1024 timesteps each:
  round k: AllReduce(k) | in_proj+conv+x_proj for round k+1 (PE/Scalar) |
           dt_proj+softplus (B) | selective scan (C, DVE-bound) |
           tiny AllToAll(k) | out_proj for t-block k (E, PE).
Output t-rows are interleaved in 128-blocks across cores so each round's
gated activations can be exchanged and out_proj'd immediately, hiding the
collective+out_proj tail under the scan. Activations (xs, silu(z), dts,
dtx) stay in SBUF rings; out_proj weights stay resident in SBUF.
DVE runs only the scan chain (exp on Scalar via act tables incl. native
Silu; PSUM->SBUF copies and casts on Scalar; gate epilogue fused via
scalar_tensor_tensor reading PSUM directly).

Shapes hardcoded for: B=2, L=4096, d_model=1024, d_inner=2048, d_state=16,
d_conv=4, dt_rank=64, f32 I/O.
"""
import numpy as np
import ml_dtypes
from contextlib import ExitStack

import concourse.bass as bass
import concourse.bacc as bacc
import concourse.tile as tile
from concourse import mybir
from concourse import bass_utils

BF = ml_dtypes.bfloat16
F32 = mybir.dt.float32
BF16 = mybir.dt.bfloat16

NCORES = 8
B, L, DM = 2, 4096, 1024
DI, DS, DC, DTR = 2048, 16, 4, 64
DL = DI // NCORES          # 256 local channels
NDH = DL // 128            # 2 d-half tiles
T = B * L                  # 8192 flattened (b, l)
TCB = 1024                 # round granularity (t per round)
NR = T // TCB              # 8 rounds
TCA = 512                  # in_proj chunk
ACT = mybir.ActivationFunctionType
ALU = mybir.AluOpType

_cached = {}


def _build():
    nc = bacc.Bacc("TRN2", target_bir_lowering=False, num_devices=NCORES)

    # ---- I/O -------------------------------------------------------------
    d_hT = nc.dram_tensor("hT", (DM, T), BF16, kind="ExternalInput")
    d_wxzT = nc.dram_tensor("wxzT", (DM, 2 * DL), BF16, kind="ExternalInput")
    d_cdiag = nc.dram_tensor("cdiag", (DC, NDH, 128, 128), BF16, kind="ExternalInput")
    d_convb = nc.dram_tensor("convb", (NDH, 128, 1), F32, kind="ExternalInput")
    d_xprojT = nc.dram_tensor("xprojT", (NDH, 128, DTR + 2 * DS), BF16, kind="ExternalInput")
    d_dtwT = nc.dram_tensor("dtwT", (DTR, DL), BF16, kind="ExternalInput")
    d_dtb = nc.dram_tensor("dtb", (NDH, 128, 1), F32, kind="ExternalInput")
    d_aneg = nc.dram_tensor("aneg", (NDH, 128, DS), F32, kind="ExternalInput")
    d_dvec = nc.dram_tensor("dvec", (NDH, 128, 1), F32, kind="ExternalInput")
    d_woutT = nc.dram_tensor("woutT", (2 * NCORES, 128, DM), BF16, kind="ExternalInput")
    d_ident = nc.dram_tensor("ident", (128, 128), BF16, kind="ExternalInput")
    d_out = nc.dram_tensor("out_slice", (NR * 128, DM), F32, kind="ExternalOutput")

    # ---- internal DRAM ---------------------------------------------------
    d_xdp = nc.dram_tensor("xdp", (NR, DTR + 2 * DS, TCB), F32, kind="Internal")
    d_xd = nc.dram_tensor("xd", (NR, DTR + 2 * DS, TCB), F32, kind="Internal",
                          addr_space="Shared")
    d_bc = nc.dram_tensor("bcrows", (2 * DS, T), BF16, kind="Internal")
    # tiny warm-up collective to absorb initial inter-core skew during A(0)
    d_ri = nc.dram_tensor("warm_i", (4, 64), F32, kind="Internal")
    d_ro = nc.dram_tensor("warm_o", (4, 64), F32, kind="Internal",
                          addr_space="Shared")
    # per-round A2A pieces: [round][target core][local ch][128 t]
    d_a2ai = nc.dram_tensor("a2ai", (NR, NCORES, DL, 128), BF16, kind="Internal")
    d_a2ao = nc.dram_tensor("a2ao", (NR, NCORES, DL, 128), BF16, kind="Internal")

    groups = [list(range(NCORES))]

    with tile.TileContext(nc) as tc, ExitStack() as ctx:
        consts = ctx.enter_context(tc.tile_pool(name="consts", bufs=1))
        arena = ctx.enter_context(tc.tile_pool(name="arena", bufs=1))
        work = ctx.enter_context(tc.tile_pool(name="work", bufs=2))
        psA = ctx.enter_context(tc.tile_pool(name="psA", bufs=2, space="PSUM"))
        psY = ctx.enter_context(tc.tile_pool(name="psY", bufs=4, space="PSUM"))
        psE = ctx.enter_context(tc.tile_pool(name="psE", bufs=2, space="PSUM"))

        # ---- constants ---------------------------------------------------
        wxz = consts.tile([128, 8, 2 * DL], BF16, tag="wxz")
        nc.sync.dma_start(out=wxz, in_=d_wxzT[:, :].rearrange("(k p) m -> p k m", p=128))
        cdg = consts.tile([128, DC, NDH, 128], BF16, tag="cdg")
        nc.sync.dma_start(
            out=cdg, in_=bass.AP(tensor=d_cdiag[:, :, :, :].tensor, offset=0,
                                 ap=[[128, 128], [NDH * 128 * 128, DC], [128 * 128, NDH], [1, 128]]))
        convb = consts.tile([128, NDH, 1], F32, tag="convb")
        nc.sync.dma_start(out=convb, in_=d_convb[:, :, :].rearrange("h p one -> p h one"))
        xprj = consts.tile([128, NDH, DTR + 2 * DS], BF16, tag="xprj")
        nc.sync.dma_start(out=xprj, in_=d_xprojT[:, :, :].rearrange("h p m -> p h m"))
        dtw = consts.tile([DTR, DL], BF16, tag="dtw")
        nc.sync.dma_start(out=dtw, in_=d_dtwT[:, :])
        dtb = consts.tile([128, NDH, 1], F32, tag="dtb")
        nc.sync.dma_start(out=dtb, in_=d_dtb[:, :, :].rearrange("h p one -> p h one"))
        aneg = consts.tile([128, NDH, DS], F32, tag="aneg")
        nc.sync.dma_start(out=aneg, in_=d_aneg[:, :, :].rearrange("h p n -> p h n"))
        dvec = consts.tile([128, NDH, 1], F32, tag="dvec")
        nc.sync.dma_start(out=dvec, in_=d_dvec[:, :, :].rearrange("h p one -> p h one"))
        ident = consts.tile([128, 128], BF16, tag="ident")
        nc.sync.dma_start(out=ident, in_=d_ident[:, :])
        wout = consts.tile([128, 2 * NCORES, DM], BF16, tag="wout")
        nc.sync.dma_start(out=wout, in_=d_woutT[:, :, :].rearrange("k p m -> p k m"))
        carry = consts.tile([128, NDH, DS], F32, tag="carry")

        # ---- SBUF activation rings --------------------------------------
        xpad = arena.tile([128, NDH, B, 3 + L], BF16, tag="xpad")
        xs_r = arena.tile([128, NDH, 3, TCB], BF16, tag="xs_r")     # ring-3 by round
        zs_r = arena.tile([128, NDH, 3, TCB], BF16, tag="zs_r")     # ring-3 (silu(z))
        dts_r = arena.tile([128, NDH, 2, TCB], BF16, tag="dts_r")   # ring-2
        dtx_r = arena.tile([128, NDH, 2, TCB], BF16, tag="dtx_r")   # ring-2

        for h in range(NDH):
            for b in range(B):
                nc.vector.memset(xpad[:, h, b, 0:3], 0.0)

        # ---- phase bodies ------------------------------------------------
        def phase_A(k):
            """in_proj + z-silu + conv-silu + x_proj partial for round k."""
            b, l0 = (k * TCB) // L, (k * TCB) % L
            r3, r2 = k % 3, k % 2
            for ci in range(TCB // TCA):  # 2 chunks of 512
                t0 = k * TCB + ci * TCA
                lc = l0 + ci * TCA
                ht = work.tile([128, 8, TCA], BF16, tag="ht")
                nc.sync.dma_start(
                    out=ht,
                    in_=bass.AP(tensor=d_hT[:, :].tensor, offset=t0,
                                ap=[[T, 128], [128 * T, 8], [1, TCA]]))
                for m in range(4):  # 0,1: x halves; 2,3: z halves
                    pxz = psA.tile([128, TCA], F32, tag="ps", name=f"pxz_{k}_{ci}_{m}")
                    for kk in range(8):
                        nc.tensor.matmul(pxz, lhsT=wxz[:, kk, m * 128:(m + 1) * 128],
                                         rhs=ht[:, kk, :], start=(kk == 0), stop=(kk == 7))
                    if m < 2:
                        nc.scalar.copy(xpad[:, m, b, 3 + lc: 3 + lc + TCA], pxz)
                    else:
                        nc.scalar.activation(
                            zs_r[:, m - 2, r3, ci * TCA:(ci + 1) * TCA], pxz, ACT.Silu)
                # conv for this chunk (xpad for it was just written)
                for h in range(NDH):
                    pc = psA.tile([128, TCA], F32, tag="ps", name=f"pc_{k}_{ci}_{h}")
                    for j in range(DC):
                        nc.tensor.matmul(pc, lhsT=cdg[:, j, h, :],
                                         rhs=xpad[:, h, b, lc + j: lc + j + TCA],
                                         start=(j == 0), stop=(j == DC - 1))
                    nc.scalar.activation(
                        xs_r[:, h, r3, ci * TCA:(ci + 1) * TCA], pc, ACT.Silu,
                        bias=convb[:, h, 0:1], scale=1.0)
                # x_proj partial for this chunk
                pxp = psA.tile([96, TCA], F32, tag="ps", name=f"pxp_{k}_{ci}")
                for h in range(NDH):
                    nc.tensor.matmul(pxp, lhsT=xprj[:, h, :],
                                     rhs=xs_r[:, h, r3, ci * TCA:(ci + 1) * TCA],
                                     start=(h == 0), stop=(h == NDH - 1))
                xpt = work.tile([96, TCA], F32, tag="xpt")
                nc.scalar.copy(xpt, pxp)
                nc.sync.dma_start(out=d_xdp[k, :, ci * TCA:(ci + 1) * TCA], in_=xpt)

        def phase_B(k):
            """dt_proj + softplus + dtx; B/C rows to bf16 DRAM for bcast."""
            t0 = k * TCB
            r3, r2 = k % 3, k % 2
            xdt = work.tile([96, TCB], F32, tag="xdt", name=f"xdt_{k}")
            nc.sync.dma_start(out=xdt, in_=d_xd[k, :, :])
            xdb = work.tile([96, TCB], BF16, tag="xdb", name=f"xdb_{k}")
            nc.scalar.copy(xdb, xdt)
            nc.sync.dma_start(out=d_bc[:, t0:t0 + TCB], in_=xdb[DTR:DTR + 2 * DS, :])
            for h in range(NDH):
                for qq in range(2):
                    pdt = psA.tile([128, 512], F32, tag="ps", name=f"pdt_{k}_{h}_{qq}")
                    nc.tensor.matmul(pdt, lhsT=dtw[:, h * 128:(h + 1) * 128],
                                     rhs=xdb[0:DTR, qq * 512:(qq + 1) * 512],
                                     start=True, stop=True)
                    spe = work.tile([128, 512], F32, tag="spe", bufs=2,
                                    name=f"spe_{k}_{h}_{qq}")
                    nc.scalar.activation(spe, pdt, ACT.Exp,
                                         bias=dtb[:, h, 0:1], scale=1.0)
                    nc.scalar.activation(dts_r[:, h, r2, qq * 512:(qq + 1) * 512],
                                         spe, ACT.Ln, bias=1.0, scale=1.0)
                nc.vector.tensor_mul(dtx_r[:, h, r2, :], dts_r[:, h, r2, :],
                                     xs_r[:, h, r3, :])

        def phase_C(k, pys):
            """selective scan for round k."""
            q0, q1 = 0, 2
            c0, W = q0 * 512, (q1 - q0) * 512
            t0c = k * TCB + c0
            r3, r2 = k % 3, k % 2
            first_in_seq = (t0c % L == 0)
            copy_carry = ((t0c + W) % L != 0)
            for j in range(DS // 2):  # n-pairs
                bbc = work.tile([128, 2, W], BF16, tag="bbc",
                                name=f"bbc_{k}_{q0}_{j}")
                cbc = work.tile([128, 2, W], BF16, tag="cbc",
                                name=f"cbc_{k}_{q0}_{j}")
                for nn in range(2):
                    n = 2 * j + nn
                    nc.sync.dma_start(
                        out=bbc[:, nn, :],
                        in_=bass.AP(tensor=d_bc[:, :].tensor, offset=n * T + t0c,
                                    ap=[[0, 128], [1, W]]))
                    nc.sync.dma_start(
                        out=cbc[:, nn, :],
                        in_=bass.AP(tensor=d_bc[:, :].tensor, offset=(DS + n) * T + t0c,
                                    ap=[[0, 128], [1, W]]))
                for h in range(NDH):
                    dA = [None, None]
                    for nn in range(2):
                        n = 2 * j + nn
                        dA[nn] = work.tile([128, W], BF16, tag="dA", bufs=3,
                                           name=f"dA_{k}_{q0}_{j}_{h}_{nn}")
                        nc.scalar.activation(dA[nn], dts_r[:, h, r2, c0:c0 + W],
                                             ACT.Exp, bias=0.0,
                                             scale=aneg[:, h, n:n + 1])
                    dtxs = dtx_r[:, h, r2, c0:c0 + W]
                    dBx = work.tile([128, 2, W], BF16, tag="dBx", bufs=1,
                                    name=f"dBx_{k}_{q0}_{j}_{h}")
                    nc.vector.tensor_mul(
                        dBx,
                        bass.AP(tensor=dtxs.tensor, offset=dtxs.offset,
                                ap=[dtxs.ap[0], [0, 2], dtxs.ap[1]]),
                        bbc)
                    hts = work.tile([128, 2, W], BF16, tag="hts", bufs=1,
                                    name=f"hts_{k}_{q0}_{j}_{h}")
                    for nn in range(2):
                        n = 2 * j + nn
                        init = 0.0 if first_in_seq else carry[:, h, n:n + 1]
                        nc.vector.tensor_tensor_scan(
                            out=hts[:, nn, :], data0=dA[nn], data1=dBx[:, nn, :],
                            initial=init, op0=ALU.mult, op1=ALU.add)
                        if copy_carry:
                            nc.vector.tensor_copy(carry[:, h, n:n + 1],
                                                  hts[:, nn, W - 1:W])
                    yp = work.tile([128, 2, W], BF16, tag="yp", bufs=1,
                                   name=f"yp_{k}_{q0}_{j}_{h}")
                    nc.vector.tensor_mul(yp, hts, cbc)
                    for nn in range(2):
                        for qq in range(q0, q1):
                            nc.tensor.matmul(
                                pys[h][qq], lhsT=ident,
                                rhs=yp[:, nn, (qq - q0) * 512:(qq - q0 + 1) * 512],
                                start=(j == 0 and nn == 0),
                                stop=(j == DS // 2 - 1 and nn == 1))
            # epilogue: ys = (xs*D + psY) * silu(z); scatter to A2A input
            # (pure-DVE: reading psY via scalar_tensor_tensor avoids a
            # Scalar-engine dependency stalling the DVE at round boundaries)
            for h in range(NDH):
                ys = work.tile([128, TCB], BF16, tag="ys", name=f"ys_{k}_{h}")
                for qq in range(2):
                    nc.vector.scalar_tensor_tensor(
                        out=ys[:, qq * 512:(qq + 1) * 512],
                        in0=xs_r[:, h, r3, qq * 512:(qq + 1) * 512],
                        scalar=dvec[:, h, 0:1],
                        in1=pys[h][qq], op0=ALU.mult, op1=ALU.add)
                nc.vector.tensor_mul(ys, ys, zs_r[:, h, r3, :])
                for jsh in range(NCORES):
                    nc.sync.dma_start(
                        out=d_a2ai[k, jsh, h * 128:(h + 1) * 128, :],
                        in_=ys[:, jsh * 128:(jsh + 1) * 128])

        def phase_E(k):
            """out_proj for my interleaved t-block of round k."""
            yblk = work.tile([128, NCORES, NDH, 128], BF16, tag="yblk")
            for iq in range(4):  # split across DMA queues
                nc.sync.dma_start(
                    out=yblk[:, 2 * iq:2 * iq + 2, :, :],
                    in_=bass.AP(tensor=d_a2ao[:, :, :, :].tensor,
                                offset=(k * NCORES + 2 * iq) * DL * 128,
                                ap=[[128, 128], [DL * 128, 2], [128 * 128, NDH], [1, 128]]))
            for fh in range(2):
                pe = psE.tile([128, 512], F32, tag="pe", name=f"pe_{k}_{fh}")
                for kt in range(2 * NCORES):
                    i, h = kt // 2, kt % 2
                    nc.tensor.matmul(pe, lhsT=yblk[:, i, h, :],
                                     rhs=wout[:, kt, fh * 512:(fh + 1) * 512],
                                     start=(kt == 0), stop=(kt == 2 * NCORES - 1))
                ot = work.tile([128, 512], F32, tag="ot", bufs=1)
                nc.scalar.copy(ot, pe)
                nc.sync.dma_start(
                    out=d_out[k * 128:(k + 1) * 128, fh * 512:(fh + 1) * 512],
                    in_=ot)

        def all_reduce(k):
            nc.gpsimd.collective_compute(
                kind="AllReduce", op=ALU.add, replica_groups=groups,
                ins=[d_xdp[k, :, :]], outs=[d_xd[k, :, :]])

        def all_to_all(k):
            nc.gpsimd.collective_compute(
                kind="AllToAll", op=ALU.bypass, replica_groups=groups,
                ins=[d_a2ai[k, :, :, :]], outs=[d_a2ao[k, :, :, :]])

        # ---- pipelined schedule -----------------------------------------
        warm = consts.tile([4, 64], F32, tag="warm")
        nc.vector.memset(warm, 0.0)
        nc.sync.dma_start(out=d_ri[:, :], in_=warm)
        nc.gpsimd.collective_compute(
            kind="AllReduce", op=ALU.add, replica_groups=groups,
            ins=[d_ri[:, :]], outs=[d_ro[:, :]])
        phase_A(0)
        all_reduce(0)
        for k in range(NR):
            pys = [[psY.tile([128, 512], F32, tag="py", name=f"pys_{k}_{h}_{qq}")
                    for qq in range(2)] for h in range(NDH)]
            if k + 1 < NR:
                phase_A(k + 1)
                all_reduce(k + 1)
            phase_B(k)
            phase_C(k, pys)
            all_to_all(k)
            phase_E(k)

    nc.compile()
    return nc


def _host_prep(inputs):
    """Per-core input maps from full inputs (layout prep + bf16 casts only)."""
    hs = np.asarray(inputs["hidden_states"], np.float32)
    wxz = np.asarray(inputs["in_proj_w"], np.float32)
    cw = np.asarray(inputs["conv_w"], np.float32)
    cb = np.asarray(inputs["conv_b"], np.float32)
    xpw = np.asarray(inputs["x_proj_w"], np.float32)
    dpw = np.asarray(inputs["dt_proj_w"], np.float32)
    dpb = np.asarray(inputs["dt_proj_b"], np.float32)
    alog = np.asarray(inputs["A_log"], np.float32)
    dv = np.asarray(inputs["D"], np.float32)
    wo = np.asarray(inputs["out_proj_w"], np.float32)

    hT = np.ascontiguousarray(hs.reshape(T, DM).T).astype(BF)
    woutT = np.ascontiguousarray(wo.T).reshape(2 * NCORES, 128, DM).astype(BF)
    ident = np.eye(128, dtype=np.float32).astype(BF)

    in_maps = []
    for i in range(NCORES):
        lo = i * DL
        sl = slice(lo, lo + DL)
        wxzT = np.ascontiguousarray(
            np.concatenate([wxz[sl], wxz[DI + lo:DI + lo + DL]], axis=0).T).astype(BF)
        cdiag = np.zeros((DC, NDH, 128, 128), np.float32)
        for j in range(DC):
            for h in range(NDH):
                np.fill_diagonal(cdiag[j, h], cw[lo + h * 128:lo + (h + 1) * 128, j])
        in_maps.append({
            "hT": hT,
            "wxzT": wxzT,
            "cdiag": cdiag.astype(BF),
            "convb": cb[sl].reshape(NDH, 128, 1),
            "xprojT": np.ascontiguousarray(xpw[:, sl].T).reshape(NDH, 128, 96).astype(BF),
            "dtwT": np.ascontiguousarray(dpw[sl].T).astype(BF),
            "dtb": dpb[sl].reshape(NDH, 128, 1),
            "aneg": (-np.exp(alog[sl])).reshape(NDH, 128, DS).astype(np.float32),
            "dvec": dv[sl].reshape(NDH, 128, 1),
            "woutT": woutT,
            "ident": ident,
        })
    return in_maps


def _run(inputs, trace=False, **kw):
    if "nc" not in _cached:
        _cached["nc"] = _build()
    nc = _cached["nc"]
    in_maps = _host_prep(inputs)
    res = bass_utils.run_bass_kernel_spmd(
        nc, in_maps, core_ids=list(range(NCORES)), trace=trace, **kw)
    # core j's out_slice rows: block k (128 rows) -> global t = k*1024 + j*128
    full = np.empty((NR, NCORES, 128, DM), np.float32)
    for j in range(NCORES):
        full[:, j] = res.results[j]["out_slice"].reshape(NR, 128, DM)
    return full.reshape(B, L, DM).astype(np.float32), res


def kernel(**inputs):
    out, _ = _run(inputs, trace=False)
    return out


# revision 33
# speedup vs baseline: 1.0262x; 1.0115x over previous
"""Trainium2 Bass kernel for a dense Mamba (selective-scan) block, SPMD over 8 NeuronCores.

Sharding: tensor-parallel over d_inner (2048 -> 256 channels/core).

v2: fully software-pipelined over 8 rounds of 1024 timesteps each:
  round k: AllReduce(k) | in_proj+conv+x_proj for round k+1 (PE/Scalar) |
           dt_proj+softplus (B) | selective scan (C, DVE-bound) |
           tiny AllToAll(k) | out_proj for t-block k (E, PE).
Output t-rows are interleaved in 128-blocks across cores so each round's
gated activations can be exchanged and out_proj'd immediately, hiding the
collective+out_proj tail under the scan. Activations (xs, silu(z), dts,
dtx) stay in SBUF rings; out_proj weights stay resident in SBUF.
DVE runs only the scan chain (exp on Scalar via act tables incl. native
Silu; PSUM->SBUF copies and casts on Scalar; gate epilogue fused via
scalar_tensor_tensor reading PSUM directly).

Shapes hardcoded for: B=2, L=4096, d_model=1024, d_inner=2048, d_state=16,
d_conv=4, dt_rank=64, f32 I/O.
"""
import numpy as np
import ml_dtypes
from contextlib import ExitStack

import concourse.bass as bass
import concourse.bacc as bacc
import concourse.tile as tile
from concourse import mybir
from concourse import bass_utils

BF = ml_dtypes.bfloat16
F32 = mybir.dt.float32
BF16 = mybir.dt.bfloat16

NCORES = 8
B, L, DM = 2, 4096, 1024
DI, DS, DC, DTR = 2048, 16, 4, 64
DL = DI // NCORES          # 256 local channels
NDH = DL // 128            # 2 d-half tiles
T = B * L                  # 8192 flattened (b, l)
TCB = 1024                 # round granularity (t per round)
NR = T // TCB              # 8 rounds
TCA = 512                  # in_proj chunk
ACT = mybir.ActivationFunctionType
ALU = mybir.AluOpType

_cached = {}


def _build():
    nc = bacc.Bacc("TRN2", target_bir_lowering=False, num_devices=NCORES)

    # ---- I/O -------------------------------------------------------------
    d_hT = nc.dram_tensor("hT", (DM, T), BF16, kind="ExternalInput")
    d_wxzT = nc.dram_tensor("wxzT", (DM, 2 * DL), BF16, kind="ExternalInput")
    d_cdiag = nc.dram_tensor("cdiag", (DC, NDH, 128, 128), BF16, kind="ExternalInput")
    d_convb = nc.dram_tensor("convb", (NDH, 128, 1), F32, kind="ExternalInput")
    d_xprojT = nc.dram_tensor("xprojT", (NDH, 128, DTR + 2 * DS), BF16, kind="ExternalInput")
    d_dtwT = nc.dram_tensor("dtwT", (DTR, DL), BF16, kind="ExternalInput")
    d_dtb = nc.dram_tensor("dtb", (NDH, 128, 1), F32, kind="ExternalInput")
    d_aneg = nc.dram_tensor("aneg", (NDH, 128, DS), F32, kind="ExternalInput")
    d_dvec = nc.dram_tensor("dvec", (NDH, 128, 1), F32, kind="ExternalInput")
    d_woutT = nc.dram_tensor("woutT", (2 * NCORES, 128, DM), BF16, kind="ExternalInput")
    d_ident = nc.dram_tensor("ident", (128, 128), BF16, kind="ExternalInput")
    d_out = nc.dram_tensor("out_slice", (NR * 128, DM), F32, kind="ExternalOutput")

    # ---- internal DRAM ---------------------------------------------------
    d_xdp = nc.dram_tensor("xdp", (NR, DTR + 2 * DS, TCB), F32, kind="Internal")
    d_xd = nc.dram_tensor("xd", (NR, DTR + 2 * DS, TCB), F32, kind="Internal",
                          addr_space="Shared")
    d_bc = nc.dram_tensor("bcrows", (2 * DS, T), BF16, kind="Internal")
    # tiny warm-up collective to absorb initial inter-core skew during A(0)
    d_ri = nc.dram_tensor("warm_i", (4, 64), F32, kind="Internal")
    d_ro = nc.dram_tensor("warm_o", (4, 64), F32, kind="Internal",
                          addr_space="Shared")
    # per-round A2A pieces: [round][target core][local ch][128 t]
    d_a2ai = nc.dram_tensor("a2ai", (NR, NCORES, DL, 128), BF16, kind="Internal")
    d_a2ao = nc.dram_tensor("a2ao", (NR, NCORES, DL, 128), BF16, kind="Internal")

    groups = [list(range(NCORES))]

    with tile.TileContext(nc) as tc, ExitStack() as ctx:
        consts = ctx.enter_context(tc.tile_pool(name="consts", bufs=1))
        arena = ctx.enter_context(tc.tile_pool(name="arena", bufs=1))
        work = ctx.enter_context(tc.tile_pool(name="work", bufs=2))
        psA = ctx.enter_context(tc.tile_pool(name="psA", bufs=2, space="PSUM"))
        psY = ctx.enter_context(tc.tile_pool(name="psY", bufs=4, space="PSUM"))
        psE = ctx.enter_context(tc.tile_pool(name="psE", bufs=2, space="PSUM"))

        # ---- constants ---------------------------------------------------
        wxz = consts.tile([128, 8, 2 * DL], BF16, tag="wxz")
        nc.sync.dma_start(out=wxz, in_=d_wxzT[:, :].rearrange("(k p) m -> p k m", p=128))
        cdg = consts.tile([128, DC, NDH, 128], BF16, tag="cdg")
        nc.sync.dma_start(
            out=cdg, in_=bass.AP(tensor=d_cdiag[:, :, :, :].tensor, offset=0,
                                 ap=[[128, 128], [NDH * 128 * 128, DC], [128 * 128, NDH], [1, 128]]))
        convb = consts.tile([128, NDH, 1], F32, tag="convb")
        nc.sync.dma_start(out=convb, in_=d_convb[:, :, :].rearrange("h p one -> p h one"))
        xprj = consts.tile([128, NDH, DTR + 2 * DS], BF16, tag="xprj")
        nc.sync.dma_start(out=xprj, in_=d_xprojT[:, :, :].rearrange("h p m -> p h m"))
        dtw = consts.tile([DTR, DL], BF16, tag="dtw")
        nc.sync.dma_start(out=dtw, in_=d_dtwT[:, :])
        dtb = consts.tile([128, NDH, 1], F32, tag="dtb")
        nc.sync.dma_start(out=dtb, in_=d_dtb[:, :, :].rearrange("h p one -> p h one"))
        aneg = consts.tile([128, NDH, DS], F32, tag="aneg")
        nc.sync.dma_start(out=aneg, in_=d_aneg[:, :, :].rearrange("h p n -> p h n"))
        dvec = consts.tile([128, NDH, 1], F32, tag="dvec")
        nc.sync.dma_start(out=dvec, in_=d_dvec[:, :, :].rearrange("h p one -> p h one"))
        ident = consts.tile([128, 128], BF16, tag="ident")
        nc.sync.dma_start(out=ident, in_=d_ident[:, :])
        wout = consts.tile([128, 2 * NCORES, DM], BF16, tag="wout")
        nc.sync.dma_start(out=wout, in_=d_woutT[:, :, :].rearrange("k p m -> p k m"))
        carry = consts.tile([128, NDH, DS], F32, tag="carry")

        # ---- SBUF activation rings --------------------------------------
        xpad = arena.tile([128, NDH, B, 3 + L], BF16, tag="xpad")
        xs_r = arena.tile([128, NDH, 3, TCB], BF16, tag="xs_r")     # ring-3 by round
        zs_r = arena.tile([128, NDH, 3, TCB], BF16, tag="zs_r")     # ring-3 (silu(z))
        dts_r = arena.tile([128, NDH, 2, TCB], BF16, tag="dts_r")   # ring-2
        dtx_r = arena.tile([128, NDH, 2, TCB], BF16, tag="dtx_r")   # ring-2

        for h in range(NDH):
            for b in range(B):
                nc.vector.memset(xpad[:, h, b, 0:3], 0.0)

        # ---- phase bodies ------------------------------------------------
        def phase_A(k):
            """in_proj + z-silu + conv-silu + x_proj partial for round k."""
            b, l0 = (k * TCB) // L, (k * TCB) % L
            r3, r2 = k % 3, k % 2
            for ci in range(TCB // TCA):  # 2 chunks of 512
                t0 = k * TCB + ci * TCA
                lc = l0 + ci * TCA
                ht = work.tile([128, 8, TCA], BF16, tag="ht")
                nc.sync.dma_start(
                    out=ht,
                    in_=bass.AP(tensor=d_hT[:, :].tensor, offset=t0,
                                ap=[[T, 128], [128 * T, 8], [1, TCA]]))
                for m in range(4):  # 0,1: x halves; 2,3: z halves
                    pxz = psA.tile([128, TCA], F32, tag="ps", name=f"pxz_{k}_{ci}_{m}")
                    for kk in range(8):
                        nc.tensor.matmul(pxz, lhsT=wxz[:, kk, m * 128:(m + 1) * 128],
                                         rhs=ht[:, kk, :], start=(kk == 0), stop=(kk == 7))
                    if m < 2:
                        nc.scalar.copy(xpad[:, m, b, 3 + lc: 3 + lc + TCA], pxz)
                    else:
                        nc.scalar.activation(
                            zs_r[:, m - 2, r3, ci * TCA:(ci + 1) * TCA], pxz, ACT.Silu)
                # conv for this chunk (xpad for it was just written)
                for h in range(NDH):
                    pc = psA.tile([128, TCA], F32, tag="ps", name=f"pc_{k}_{ci}_{h}")
                    for j in range(DC):
                        nc.tensor.matmul(pc, lhsT=cdg[:, j, h, :],
                                         rhs=xpad[:, h, b, lc + j: lc + j + TCA],
                                         start=(j == 0), stop=(j == DC - 1))
                    nc.scalar.activation(
                        xs_r[:, h, r3, ci * TCA:(ci + 1) * TCA], pc, ACT.Silu,
                        bias=convb[:, h, 0:1], scale=1.0)
                # x_proj partial for this chunk
                pxp = psA.tile([96, TCA], F32, tag="ps", name=f"pxp_{k}_{ci}")
                for h in range(NDH):
                    nc.tensor.matmul(pxp, lhsT=xprj[:, h, :],
                                     rhs=xs_r[:, h, r3, ci * TCA:(ci + 1) * TCA],
                                     start=(h == 0), stop=(h == NDH - 1))
                xpt = work.tile([96, TCA], F32, tag="xpt")
                nc.scalar.copy(xpt, pxp)
                nc.sync.dma_start(out=d_xdp[k, :, ci * TCA:(ci + 1) * TCA], in_=xpt)

        def phase_B(k):
            """dt_proj + softplus + dtx; B/C rows to bf16 DRAM for bcast."""
            t0 = k * TCB
            r3, r2 = k % 3, k % 2
            xdt = work.tile([96, TCB], F32, tag="xdt", name=f"xdt_{k}")
            nc.sync.dma_start(out=xdt, in_=d_xd[k, :, :])
            xdb = work.tile([96, TCB], BF16, tag="xdb", name=f"xdb_{k}")
            nc.scalar.copy(xdb, xdt)
            nc.sync.dma_start(out=d_bc[:, t0:t0 + TCB], in_=xdb[DTR:DTR + 2 * DS, :])
            for h in range(NDH):
                for qq in range(2):
                    pdt = psA.tile([128, 512], F32, tag="ps", name=f"pdt_{k}_{h}_{qq}")
                    nc.tensor.matmul(pdt, lhsT=dtw[:, h * 128:(h + 1) * 128],
                                     rhs=xdb[0:DTR, qq * 512:(qq + 1) * 512],
                                     start=True, stop=True)
                    spe = work.tile([128, 512], F32, tag="spe", bufs=2,
                                    name=f"spe_{k}_{h}_{qq}")
                    nc.scalar.activation(spe, pdt, ACT.Exp,
                                         bias=dtb[:, h, 0:1], scale=1.0)
                    nc.scalar.activation(dts_r[:, h, r2, qq * 512:(qq + 1) * 512],
                                         spe, ACT.Ln, bias=1.0, scale=1.0)
                nc.vector.tensor_mul(dtx_r[:, h, r2, :], dts_r[:, h, r2, :],
                                     xs_r[:, h, r3, :])

        def phase_C(k, pys):
            """selective scan for round k."""
            q0, q1 = 0, 2
            c0, W = q0 * 512, (q1 - q0) * 512
            t0c = k * TCB + c0
            r3, r2 = k % 3, k % 2
            first_in_seq = (t0c % L == 0)
            copy_carry = ((t0c + W) % L != 0)
            for j in range(DS // 2):  # n-pairs
                bbc = work.tile([128, 2, W], BF16, tag="bbc",
                                name=f"bbc_{k}_{q0}_{j}")
                cbc = work.tile([128, 2, W], BF16, tag="cbc",
                                name=f"cbc_{k}_{q0}_{j}")
                for nn in range(2):
                    n = 2 * j + nn
                    nc.sync.dma_start(
                        out=bbc[:, nn, :],
                        in_=bass.AP(tensor=d_bc[:, :].tensor, offset=n * T + t0c,
                                    ap=[[0, 128], [1, W]]))
                    nc.sync.dma_start(
                        out=cbc[:, nn, :],
                        in_=bass.AP(tensor=d_bc[:, :].tensor, offset=(DS + n) * T + t0c,
                                    ap=[[0, 128], [1, W]]))
                for h in range(NDH):
                    dA = [None, None]
                    for nn in range(2):
                        n = 2 * j + nn
                        dA[nn] = work.tile([128, W], BF16, tag="dA", bufs=3,
                                           name=f"dA_{k}_{q0}_{j}_{h}_{nn}")
                        nc.scalar.activation(dA[nn], dts_r[:, h, r2, c0:c0 + W],
                                             ACT.Exp, bias=0.0,
                                             scale=aneg[:, h, n:n + 1])
                    dtxs = dtx_r[:, h, r2, c0:c0 + W]
                    dBx = work.tile([128, 2, W], BF16, tag="dBx", bufs=1,
                                    name=f"dBx_{k}_{q0}_{j}_{h}")
                    nc.vector.tensor_mul(
                        dBx,
                        bass.AP(tensor=dtxs.tensor, offset=dtxs.offset,
                                ap=[dtxs.ap[0], [0, 2], dtxs.ap[1]]),
                        bbc)
                    hts = work.tile([128, 2, W], BF16, tag="hts", bufs=1,
                                    name=f"hts_{k}_{q0}_{j}_{h}")
                    for nn in range(2):
                        n = 2 * j + nn
                        init = 0.0 if first_in_seq else carry[:, h, n:n + 1]
                        nc.vector.tensor_tensor_scan(
                            out=hts[:, nn, :], data0=dA[nn], data1=dBx[:, nn, :],
                            initial=init, op0=ALU.mult, op1=ALU.add)
                        if copy_carry:
                            nc.vector.tensor_copy(carry[:, h, n:n + 1],
                                                  hts[:, nn, W - 1:W])
                    yp = work.tile([128, 2, W], BF16, tag="yp", bufs=1,
                                   name=f"yp_{k}_{q0}_{j}_{h}")
                    nc.vector.tensor_mul(yp, hts, cbc)
                    for nn in range(2):
                        for qq in range(q0, q1):
                            nc.tensor.matmul(
                                pys[h][qq], lhsT=ident,
                                rhs=yp[:, nn, (qq - q0) * 512:(qq - q0 + 1) * 512],
                                start=(j == 0 and nn == 0),
                                stop=(j == DS // 2 - 1 and nn == 1))
            # epilogue: ys = (xs*D + psY) * silu(z); scatter to A2A input
            # (pure-DVE: reading psY via scalar_tensor_tensor avoids a
            # Scalar-engine dependency stalling the DVE at round boundaries)
            for h in range(NDH):
                ys = work.tile([128, TCB], BF16, tag="ys", name=f"ys_{k}_{h}")
                for qq in range(2):
                    nc.vector.scalar_tensor_tensor(
                        out=ys[:, qq * 512:(qq + 1) * 512],
                        in0=xs_r[:, h, r3, qq * 512:(qq + 1) * 512],
                        scalar=dvec[:, h, 0:1],
                        in1=pys[h][qq], op0=ALU.mult, op1=ALU.add)
                nc.vector.tensor_mul(ys, ys, zs_r[:, h, r3, :])
                for jsh in range(NCORES):
                    nc.sync.dma_start(
                        out=d_a2ai[k, jsh, h * 128:(h + 1) * 128, :],
                        in_=ys[:, jsh * 128:(jsh + 1) * 128])

        def phase_E(k):
            """out_proj for my interleaved t-block of round k."""
            yblk = work.tile([128, NCORES, NDH, 128], BF16, tag="yblk")
            for iq in range(4):  # split across DMA queues
                nc.sync.dma_start(
                    out=yblk[:, 2 * iq:2 * iq + 2, :, :],
                    in_=bass.AP(tensor=d_a2ao[:, :, :, :].tensor,
                                offset=(k * NCORES + 2 * iq) * DL * 128,
                                ap=[[128, 128], [DL * 128, 2], [128 * 128, NDH], [1, 128]]))
            for fh in range(2):
                pe = psE.tile([128, 512], F32, tag="pe", name=f"pe_{k}_{fh}")
                for kt in range(2 * NCORES):
                    i, h = kt // 2, kt % 2
                    nc.tensor.matmul(pe, lhsT=yblk[:, i, h, :],
                                     rhs=wout[:, kt, fh * 512:(fh + 1) * 512],
                                     start=(kt == 0), stop=(kt == 2 * NCORES - 1))
                ot = work.tile([128, 512], F32, tag="ot", bufs=1)
                nc.scalar.copy(ot, pe)
                nc.sync.dma_start(
                    out=d_out[k * 128:(k + 1) * 128, fh * 512:(fh + 1) * 512],
                    in_=ot)

        def all_reduce(k):
            nc.gpsimd.collective_compute(
                kind="AllReduce", op=ALU.add, replica_groups=groups,
                ins=[d_xdp[k, :, :]], outs=[d_xd[k, :, :]])

        def all_to_all(k):
            nc.gpsimd.collective_compute(
                kind="AllToAll", op=ALU.bypass, replica_groups=groups,
                ins=[d_a2ai[k, :, :, :]], outs=[d_a2ao[k, :, :, :]])

        # ---- pipelined schedule -----------------------------------------
        phase_A(0)
        all_reduce(0)
        for k in range(NR):
            pys = [[psY.tile([128, 512], F32, tag="py", name=f"pys_{k}_{h}_{qq}")
                    for qq in range(2)] for h in range(NDH)]
            if k + 1 < NR:
                phase_A(k + 1)
                all_reduce(k + 1)
            phase_B(k)
            phase_C(k, pys)
            all_to_all(k)
            phase_E(k)

    nc.compile()
    return nc


def _host_prep(inputs):
    """Per-core input maps from full inputs (layout prep + bf16 casts only)."""
    hs = np.asarray(inputs["hidden_states"], np.float32)
    wxz = np.asarray(inputs["in_proj_w"], np.float32)
    cw = np.asarray(inputs["conv_w"], np.float32)
    cb = np.asarray(inputs["conv_b"], np.float32)
    xpw = np.asarray(inputs["x_proj_w"], np.float32)
    dpw = np.asarray(inputs["dt_proj_w"], np.float32)
    dpb = np.asarray(inputs["dt_proj_b"], np.float32)
    alog = np.asarray(inputs["A_log"], np.float32)
    dv = np.asarray(inputs["D"], np.float32)
    wo = np.asarray(inputs["out_proj_w"], np.float32)

    hT = np.ascontiguousarray(hs.reshape(T, DM).T).astype(BF)
    woutT = np.ascontiguousarray(wo.T).reshape(2 * NCORES, 128, DM).astype(BF)
    ident = np.eye(128, dtype=np.float32).astype(BF)

    in_maps = []
    for i in range(NCORES):
        lo = i * DL
        sl = slice(lo, lo + DL)
        wxzT = np.ascontiguousarray(
            np.concatenate([wxz[sl], wxz[DI + lo:DI + lo + DL]], axis=0).T).astype(BF)
        cdiag = np.zeros((DC, NDH, 128, 128), np.float32)
        for j in range(DC):
            for h in range(NDH):
                np.fill_diagonal(cdiag[j, h], cw[lo + h * 128:lo + (h + 1) * 128, j])
        in_maps.append({
            "hT": hT,
            "wxzT": wxzT,
            "cdiag": cdiag.astype(BF),
            "convb": cb[sl].reshape(NDH, 128, 1),
            "xprojT": np.ascontiguousarray(xpw[:, sl].T).reshape(NDH, 128, 96).astype(BF),
            "dtwT": np.ascontiguousarray(dpw[sl].T).astype(BF),
            "dtb": dpb[sl].reshape(NDH, 128, 1),
            "aneg": (-np.exp(alog[sl])).reshape(NDH, 128, DS).astype(np.float32),
            "dvec": dv[sl].reshape(NDH, 128, 1),
            "woutT": woutT,
            "ident": ident,
        })
    return in_maps


def _run(inputs, trace=False, **kw):
    if "nc" not in _cached:
        _cached["nc"] = _build()
    nc = _cached["nc"]
    in_maps = _host_prep(inputs)
    res = bass_utils.run_bass_kernel_spmd(
        nc, in_maps, core_ids=list(range(NCORES)), trace=trace, **kw)
    # core j's out_slice rows: block k (128 rows) -> global t = k*1024 + j*128
    full = np.empty((NR, NCORES, 128, DM), np.float32)
    for j in range(NCORES):
        full[:, j] = res.results[j]["out_slice"].reshape(NR, 128, DM)
    return full.reshape(B, L, DM).astype(np.float32), res


def kernel(**inputs):
    out, _ = _run(inputs, trace=False)
    return out
